# revision 12
# baseline (speedup 1.0000x reference)
"""Trainium2 Bass kernel for nn_Block_49624052138029 (dense transformer block).

Strategy: data parallel over 8 NeuronCores; core i owns batch i//4 and query
chunk i%4 (512 tokens, permuted to the front host-side). K/V are computed
redundantly over all 2048 tokens of the core's batch (cheaper than on-chip
collectives on this stack).

v2 layout: everything feature-major ([feature on partitions, tokens on free])
with ZERO on-chip transposes in the hot path:
 - x arrives from host three ways: token-major bf16 (LayerNorm statistics
   only), feature-major fp8 (matmul moving operand), own-chunk feature-major
   f32 (residual accumulator).
 - LayerNorm 1 is applied *virtually*: the projections consume raw fp8 x^T
   plus one augmented contraction row (mov = mu*rstd, stat = -colsum(W)) that
   subtracts the mean inside PSUM, and the rstd scale is applied at PSUM
   evacuation (TT against an rstd broadcast built by a PE ones-matmul).
 - All dense matmuls (QKV/Wo/FFN) run fp8 DoubleRow (weights scaled x16 into
   fp8's normal range; compensated at evacuation). Contraction 1024 = 4 DR
   chunks instead of 8 bf16 chunks.
 - QK^T is row-tiled: head pair (2p, 2p+1) lives on partitions 0:64 / 64:128
   of feature chunk p, and the two 64-contraction matmuls run concurrently in
   the PE array via tile_position=(64,0) for the upper head.
 - Softmax: exp on ScalarE -> fp8, AV via fp8 DoubleRow with an appended
   ones-column so the denominator falls out of the same accumulation; the
   reciprocal 1/D = exp(-ln(D)) runs on ScalarE (DVE reciprocal on a
   single-partition row measured ~2.7us each).
 - LN2 statistics via PE ones-column matmuls on the feature-major residual
   (sum and sum-of-squares), row math on [1,512] tiles, normalized h2
   materialized fp8 for the FFN.
Output is written feature-major and transposed on the host.
"""

import os
import sys

for _p in ("/root/.axon_site", "/root/.axon_site/_ro/trn_rl_repo",
           "/root/.axon_site/_ro/pypackages", "/opt/trn_rl_repo", "/opt/pypackages"):
    if os.path.isdir(_p) and _p not in sys.path:
        sys.path.append(_p)

import numpy as np
import ml_dtypes

import concourse.bass as bass
import concourse.tile as tile
from concourse import bacc, mybir
from concourse.bass_utils import run_bass_kernel_spmd
from concourse.masks import make_identity

F32 = mybir.dt.float32
BF16 = mybir.dt.bfloat16
FP8 = mybir.dt.float8e4
AF = mybir.ActivationFunctionType
ALU = mybir.AluOpType
DR = mybir.MatmulPerfMode.DoubleRow

NP_F8 = ml_dtypes.float8_e4m3
NP_BF = ml_dtypes.bfloat16

D = 1024
H = 16
E = 64
T = 2048
TQ = 512
P = 128
KO = 8            # 128-feature chunks
KP = 4            # 256-feature DoubleRow chunks
SO = 16           # key chunks per batch
EPS = 1e-5
SW = 16.0         # fp8 weight scale
# scores carry x16 from q and x16 from k
SCALE_EXP = (D ** -0.5) / (SW * SW)


def build_kernel():
    nc = bacc.Bacc(None, target_bir_lowering=False, debug=False)

    xrows = nc.dram_tensor("xrows", [T, D], BF16, kind="ExternalInput")
    xt8 = nc.dram_tensor("xt8", [D, T], FP8, kind="ExternalInput")
    xt0 = nc.dram_tensor("xt0", [D, TQ], F32, kind="ExternalInput")
    wq8 = nc.dram_tensor("wq8", [D, D], FP8, kind="ExternalInput")
    wk8 = nc.dram_tensor("wk8", [D, D], FP8, kind="ExternalInput")
    wv8 = nc.dram_tensor("wv8", [D, D], FP8, kind="ExternalInput")
    wo8 = nc.dram_tensor("wo8", [D, D], FP8, kind="ExternalInput")
    w18 = nc.dram_tensor("w18", [D, D], BF16, kind="ExternalInput")
    w28 = nc.dram_tensor("w28", [D, D], BF16, kind="ExternalInput")
    aug_q = nc.dram_tensor("aug_q", [2, D], BF16, kind="ExternalInput")
    aug_k = nc.dram_tensor("aug_k", [2, D], BF16, kind="ExternalInput")
    aug_v = nc.dram_tensor("aug_v", [2, D], BF16, kind="ExternalInput")
    bo = nc.dram_tensor("bo", [D], F32, kind="ExternalInput")
    b1 = nc.dram_tensor("b1", [D], F32, kind="ExternalInput")
    b2 = nc.dram_tensor("b2", [D], F32, kind="ExternalInput")
    out = nc.dram_tensor("out", [D, TQ], F32, kind="ExternalOutput")
    scratch = nc.dram_tensor("scratch", [2 * 24 * P], BF16, kind="Internal")

    def pm(v):                      # [D] -> [128, 8] per-partition layout
        return v.rearrange("(o p) -> p o", p=P)

    def wdr(w):                     # [D, D] -> [p, a, t, n] DR stationary view
        return w.rearrange("(a t p) n -> p a t n", t=2, p=P)

    with tile.TileContext(nc) as tc:
        with (
            tc.tile_pool(name="singles", bufs=1) as singles,
            tc.tile_pool(name="persist", bufs=1) as persist,
            tc.tile_pool(name="xrow", bufs=3) as xrow_pool,
            tc.tile_pool(name="stats", bufs=6) as stats_pool,
            tc.tile_pool(name="lnrow", bufs=2) as lnrow_pool,
            tc.tile_pool(name="ln2row", bufs=1) as ln2row_pool,
            tc.tile_pool(name="evac", bufs=3) as evac_pool,
            tc.tile_pool(name="dbc", bufs=2) as dbc_pool,
            tc.tile_pool(name="wsmall", bufs=3) as wsmall_pool,
            tc.tile_pool(name="wstrip", bufs=2) as wstrip_pool,
            tc.tile_pool(name="exps", bufs=6) as exps_pool,
            tc.tile_pool(name="ps_w", bufs=2, space="PSUM") as ps_w,
            tc.tile_pool(name="ps_qk", bufs=2, space="PSUM") as ps_qk,
            tc.tile_pool(name="ps_u", bufs=2, space="PSUM") as ps_u,
        ):
            # ---------------- setup ----------------
            id_f32 = singles.tile([P, P], F32, name="id_f32")
            make_identity(nc, id_f32[:])
            ones64 = singles.tile([1, E], BF16, name="ones64")
            nc.vector.memset(ones64[:], 1.0)
            ones1x128 = singles.tile([1, P], BF16, name="ones1x128")
            nc.vector.memset(ones1x128[:], 1.0)
            onescol_f32 = singles.tile([P, 1], F32, name="onescol_f32")
            nc.vector.memset(onescol_f32[:], 1.0)
            eps_t = singles.tile([P, 1], F32, name="eps_t")
            nc.vector.memset(eps_t[:], EPS)

            bo_pm = singles.tile([P, KO], F32, name="bo_pm")
            nc.sync.dma_start(bo_pm[:], pm(bo))
            b1_pm = singles.tile([P, KO], F32, name="b1_pm")
            nc.sync.dma_start(b1_pm[:], pm(b1))
            b2_pm = singles.tile([P, KO], F32, name="b2_pm")
            nc.sync.dma_start(b2_pm[:], pm(b2))
            aq_sb = singles.tile([2, D], BF16, name="aq_sb")
            nc.sync.dma_start(aq_sb[:], aug_q[0:2, :])
            ak_sb = singles.tile([2, D], BF16, name="ak_sb")
            nc.sync.dma_start(ak_sb[:], aug_k[0:2, :])
            av_sb = singles.tile([2, D], BF16, name="av_sb")
            nc.sync.dma_start(av_sb[:], aug_v[0:2, :])

            # ---------------- persistent tiles ----------------
            xT8 = persist.tile([P, KO, T], FP8, name="xT8")        # 2 MB
            nc.sync.dma_start(xT8[:], xt8.rearrange("(o p) t -> p o t", p=P))
            sall = persist.tile([P, SO, 3], F32, name="sall")      # s, std, rstd
            rows_bf = persist.tile([2, T], BF16, name="rows_bf")
            rstd_row = persist.tile([1, T], BF16, name="rstd_row")
            rstd_bc = persist.tile([P, T], F32, name="rstd_bc")    # 1 MB
            kT8 = persist.tile([P, KO, T], BF16, name="kT8")       # 4 MB
            qT8 = persist.tile([P, KO, TQ], BF16, name="qT8")
            vP = persist.tile([P, SO // 2, 2, H, E + 1], FP8, name="vP")
            nc.vector.memset(vP[:, :, :, :, E], 1.0)
            oT8 = persist.tile([P, KO, TQ], FP8, name="oT8")
            x1T = persist.tile([P, KO, TQ], F32, name="x1T")       # 2 MB
            nc.sync.dma_start(x1T[:], xt0.rearrange("(o p) t -> p o t", p=P))
            h2T8 = persist.tile([P, KO, TQ], BF16, name="h2T8")
            fT8 = persist.tile([P, KO, TQ], BF16, name="fT8")

            # ---------------- LN1 statistics (two halves of 8 row-tiles) ----
            for half in range(2):
                for r in range(8):
                    rt = half * 8 + r
                    xr = xrow_pool.tile([P, D], BF16, tag="xr")
                    nc.sync.dma_start(xr[:], xrows[rt * P:(rt + 1) * P, :])
                    st = stats_pool.tile([P, 2, 6], F32, tag="bnstats")
                    xg = xr[:].rearrange("p (g d) -> p g d", g=2)
                    for g in range(2):
                        nc.vector.bn_stats(out=st[:, g, :], in_=xg[:, g, :])
                    mv = stats_pool.tile([P, 2], F32, tag="bnaggr")
                    nc.vector.bn_aggr(out=mv[:], in_=st[:])
                    # std = sqrt(var + eps); rstd = 1/std; s = mean*rstd
                    nc.scalar.activation(out=sall[:, rt, 1:2], in_=mv[:, 1:2],
                                         func=AF.Sqrt, bias=eps_t[:], scale=1.0)
                    nc.vector.reciprocal(out=sall[:, rt, 2:3],
                                         in_=sall[:, rt, 1:2])
                    nc.vector.tensor_tensor(
                        out=sall[:, rt, 0:1], in0=mv[:, 0:1],
                        in1=sall[:, rt, 2:3], op=ALU.mult)
                # transpose this half's stats into free-dim rows
                trp = ps_w.tile([P, TQ], F32, tag="ps_w")
                nc.tensor.transpose(
                    trp[0:24, 0:P],
                    sall[:, half * 8:(half + 1) * 8, :].rearrange(
                        "p a b -> p (a b)"),
                    id_f32[:])
                stg = xrow_pool.tile([24, P], BF16, tag="stg")
                nc.vector.tensor_copy(out=stg[:], in_=trp[0:24, 0:P])
                nc.sync.dma_start(
                    scratch.rearrange("(h q n) -> h q n", q=24, n=P)[half],
                    stg[:])
                scr = scratch.rearrange("(h a k n) -> h k a n", h=2, k=3, n=P)
                nc.sync.dma_start(
                    rows_bf[0:2, half * 1024:(half + 1) * 1024].rearrange(
                        "o (a n) -> o a n", n=P), scr[half, 0:2])
                nc.sync.dma_start(
                    rstd_row[0:1, half * 1024:(half + 1) * 1024].rearrange(
                        "o (a n) -> o a n", n=P), scr[half, 2:3])
                # rstd broadcast for this half via PE ones-matmul
                pb = ps_qk.tile([P, 2, TQ], F32, tag="ps_qk")
                for hh in range(2):
                    base = half * 1024 + hh * TQ
                    nc.tensor.matmul(pb[:, hh, :], ones1x128[:],
                                     rstd_row[0:1, base:base + TQ],
                                     start=True, stop=True)
                nc.vector.tensor_copy(
                    out=rstd_bc[:, half * 1024:(half + 1) * 1024],
                    in_=pb[:].rearrange("p a b -> p (a b)"))

            # ---------------- dense projection emitters ----------------
            def emit_q(he):
                wq_he = wsmall_pool.tile([P, KP, 2, P], FP8, tag="w_he")
                nc.sync.dma_start(wq_he[:], wdr(wq8)[:, :, :, he * P:(he + 1) * P])
                psq = ps_w.tile([P, TQ], F32, tag="ps_w")
                for a in range(KP):
                    nc.tensor.matmul(psq[:], wq_he[:, a], xT8[:, 2 * a:2 * a + 2, 0:TQ],
                                     start=(a == 0), stop=False, perf_mode=DR)
                nc.tensor.matmul(psq[:], aq_sb[:, he * P:(he + 1) * P],
                                 rows_bf[0:2, 0:TQ], start=False, stop=True)
                nc.vector.tensor_tensor(out=qT8[:, he, :], in0=psq[:],
                                        in1=rstd_bc[:, 0:TQ], op=ALU.mult)

            def emit_k(he, t_list=None):
                wk_he = wsmall_pool.tile([P, KP, 2, P], FP8, tag="w_he")
                nc.sync.dma_start(wk_he[:], wdr(wk8)[:, :, :, he * P:(he + 1) * P])
                for t in (range(4) if t_list is None else t_list):
                    tsl = slice(t * TQ, (t + 1) * TQ)
                    psk = ps_w.tile([P, TQ], F32, tag="ps_w")
                    for a in range(KP):
                        nc.tensor.matmul(psk[:], wk_he[:, a], xT8[:, 2 * a:2 * a + 2, tsl],
                                         start=(a == 0), stop=False, perf_mode=DR)
                    nc.tensor.matmul(psk[:], ak_sb[:, he * P:(he + 1) * P],
                                     rows_bf[0:2, tsl], start=False, stop=True)
                    nc.vector.tensor_tensor(out=kT8[:, he, tsl], in0=psk[:],
                                            in1=rstd_bc[:, tsl], op=ALU.mult)

            def emit_v_load(nh):
                wv_strip = wstrip_pool.tile([P, KP, 2, TQ], FP8, tag="wstrip")
                nc.sync.dma_start(
                    wv_strip[:], wdr(wv8)[:, :, :, nh * TQ:(nh + 1) * TQ])
                return wv_strip

            def emit_v(nh, wv_strip=None, so_list=None):
                if wv_strip is None:
                    wv_strip = emit_v_load(nh)
                for so in (range(SO) if so_list is None else so_list):
                    ssl = slice(so * P, (so + 1) * P)
                    psv = ps_w.tile([P, TQ], F32, tag="ps_w")
                    for a in range(KP):
                        nc.tensor.matmul(psv[:], xT8[:, 2 * a:2 * a + 2, ssl],
                                         wv_strip[:, a], start=(a == 0),
                                         stop=False, perf_mode=DR)
                    nc.tensor.matmul(psv[:], rows_bf[0:2, ssl],
                                     av_sb[:, nh * TQ:(nh + 1) * TQ],
                                     start=False, stop=True)
                    nc.vector.tensor_scalar(
                        out=vP[:, so // 2, so % 2, nh * 8:(nh + 1) * 8, 0:E],
                        in0=psv[:].rearrange("p (h e) -> p h e", e=E),
                        scalar1=sall[:, so, 2:3], scalar2=None, op0=ALU.mult)

            # ---------------- attention ----------------
            def emit_attn(pair):
                ha, hb = 2 * pair, 2 * pair + 1
                psuA = ps_u.tile([P, TQ], F32, tag="ps_u", name="psuA")
                psuB = ps_u.tile([P, TQ], F32, tag="ps_u", name="psuB")
                for sp in range(SO // 2):
                    pssA = ps_qk.tile([P, 2, TQ], F32, tag="ps_qk", name="pssA")
                    pssB = ps_qk.tile([P, 2, TQ], F32, tag="ps_qk", name="pssB")
                    for j in range(2):
                        so = 2 * sp + j
                        ssl = slice(so * P, (so + 1) * P)
                        nc.tensor.matmul(pssA[:, j, :], kT8[0:E, pair, ssl],
                                         qT8[0:E, pair, :], start=True, stop=True)
                        nc.tensor.matmul(pssB[:, j, :], kT8[E:P, pair, ssl],
                                         qT8[E:P, pair, :], start=True, stop=True,
                                         tile_position=(E, 0))
                    esA = exps_pool.tile([P, 2, TQ], FP8, tag="exps", name="esA")
                    nc.scalar.activation(out=esA[:], in_=pssA[:], func=AF.Exp,
                                         scale=SCALE_EXP)
                    esB = exps_pool.tile([P, 2, TQ], FP8, tag="exps", name="esB")
                    nc.scalar.activation(out=esB[:], in_=pssB[:], func=AF.Exp,
                                         scale=SCALE_EXP)
                    nc.tensor.matmul(psuA[0:E + 1, :], vP[:, sp, :, ha, :], esA[:],
                                     start=(sp == 0), stop=(sp == SO // 2 - 1),
                                     perf_mode=DR)
                    nc.tensor.matmul(psuB[0:E + 1, :], vP[:, sp, :, hb, :], esB[:],
                                     start=(sp == 0), stop=(sp == SO // 2 - 1),
                                     perf_mode=DR)
                # 1/D = exp(-ln(D)) on ScalarE, straight off the PSUM row
                dinvs = []
                for psu in (psuA, psuB):
                    lnd = lnrow_pool.tile([1, TQ], F32, tag="lnd")
                    nc.scalar.activation(out=lnd[:], in_=psu[E:E + 1, :], func=AF.Ln)
                    dinv = lnrow_pool.tile([1, TQ], BF16, tag="dinv")
                    nc.scalar.activation(out=dinv[:], in_=lnd[:], func=AF.Exp,
                                         scale=-1.0)
                    dinvs.append(dinv)

                def epi():
                    for side, (psu, dinv) in enumerate(zip((psuA, psuB), dinvs)):
                        psb = ps_w.tile([P, TQ], F32, tag="ps_w")
                        nc.tensor.matmul(psb[0:E, :], ones64[:], dinv[:],
                                         start=True, stop=True)
                        dbc = dbc_pool.tile([E, TQ], BF16, tag="dbc")
                        nc.vector.tensor_copy(out=dbc[:], in_=psb[0:E, :])
                        nc.vector.tensor_tensor(
                            out=oT8[side * E:(side + 1) * E, pair, :],
                            in0=psu[0:E, :], in1=dbc[:], op=ALU.mult)
                return epi

            for he in range(KO):
                emit_q(he)
            emit_k(0)
            emit_v(0)
            pend = None
            v1_strip = None
            for pair in range(KO):
                epi = emit_attn(pair)
                # dense interlude keeps PE fed while ACT drains the last exps
                if pair + 1 < KO:
                    emit_k(pair + 1)
                if pair == 2:
                    v1_strip = emit_v_load(1)
                    emit_v(1, v1_strip, list(range(0, 8)))
                elif pair == 3:
                    emit_v(1, v1_strip, list(range(8, SO)))
                epi()

            # ---------------- Wo projection + residual ----------------
            for half in range(2):
                wo_strip = wstrip_pool.tile([P, KP, 2, TQ], FP8, tag="wstrip")
                nc.sync.dma_start(
                    wo_strip[:], wdr(wo8)[:, :, :, half * TQ:(half + 1) * TQ])
                for m in range(4):
                    mm = half * 4 + m
                    psy = ps_w.tile([P, TQ], F32, tag="ps_w")
                    for a in range(KP):
                        nc.tensor.matmul(
                            psy[:], wo_strip[:, a, :, m * P:(m + 1) * P],
                            oT8[:, 2 * a:2 * a + 2, :],
                            start=(a == 0), stop=(a == KP - 1), perf_mode=DR)
                    ybf = evac_pool.tile([P, TQ], BF16, tag="ybf")
                    nc.vector.tensor_scalar(
                        out=ybf[:], in0=psy[:], scalar1=1.0 / (SW * SW),
                        scalar2=bo_pm[:, mm:mm + 1], op0=ALU.mult, op1=ALU.add)
                    nc.vector.tensor_tensor(out=x1T[:, mm, :], in0=x1T[:, mm, :],
                                            in1=ybf[:], op=ALU.add)

            # ---------------- LN2 (feature-major, PE column sums) ----------
            psS = ps_w.tile([P, TQ], F32, tag="ps_w")
            for ko in range(KO):
                nc.tensor.matmul(psS[0:1, :], onescol_f32[:], x1T[:, ko, :],
                                 start=(ko == 0), stop=(ko == KO - 1))
            psQ2 = ps_w.tile([P, TQ], F32, tag="ps_w")
            for ko in range(KO):
                sq = evac_pool.tile([P, TQ], F32, tag="sq")
                nc.vector.tensor_tensor(out=sq[:], in0=x1T[:, ko, :],
                                        in1=x1T[:, ko, :], op=ALU.mult)
                nc.tensor.matmul(psQ2[0:1, :], onescol_f32[:], sq[:],
                                 start=(ko == 0), stop=(ko == KO - 1))
            mu2 = ln2row_pool.tile([1, TQ], F32, tag="mu2")
            nc.vector.tensor_scalar(out=mu2[:], in0=psS[0:1, :], scalar1=1.0 / D,
                                    scalar2=None, op0=ALU.mult)
            m2 = ln2row_pool.tile([1, TQ], F32, tag="m2")
            nc.vector.tensor_scalar(out=m2[:], in0=psQ2[0:1, :], scalar1=1.0 / D,
                                    scalar2=None, op0=ALU.mult)
            var2 = ln2row_pool.tile([1, TQ], F32, tag="var2")
            nc.vector.tensor_tensor(out=var2[:], in0=mu2[:], in1=mu2[:],
                                    op=ALU.mult)
            nc.vector.tensor_tensor(out=var2[:], in0=m2[:], in1=var2[:],
                                    op=ALU.subtract)
            lnv = ln2row_pool.tile([1, TQ], F32, tag="lnv")
            nc.scalar.activation(out=lnv[:], in_=var2[:], func=AF.Ln,
                                 bias=eps_t[0:1, :])
            rstd2 = ln2row_pool.tile([1, TQ], BF16, tag="rstd2")
            nc.scalar.activation(out=rstd2[:], in_=lnv[:], func=AF.Exp, scale=-0.5)
            rstd2f = ln2row_pool.tile([1, TQ], F32, tag="rstd2f")
            nc.vector.tensor_copy(out=rstd2f[:], in_=rstd2[:])
            s2 = ln2row_pool.tile([1, TQ], BF16, tag="s2")
            nc.vector.tensor_tensor(out=s2[:], in0=mu2[:], in1=rstd2f[:],
                                    op=ALU.mult)
            psR = ps_w.tile([P, TQ], F32, tag="ps_w")
            nc.tensor.matmul(psR[:], ones1x128[:], rstd2[:], start=True, stop=True)
            psM = ps_w.tile([P, TQ], F32, tag="ps_w")
            nc.tensor.matmul(psM[:], ones1x128[:], s2[:], start=True, stop=True)
            for ko in range(KO):
                t1 = evac_pool.tile([P, TQ], BF16, tag="t1")
                nc.vector.tensor_tensor(out=t1[:], in0=x1T[:, ko, :], in1=psR[:],
                                        op=ALU.mult)
                nc.vector.tensor_tensor(out=h2T8[:, ko, :], in0=t1[:], in1=psM[:],
                                        op=ALU.subtract)

            # ---------------- FFN ----------------
            for half in range(2):
                w1_strip = wstrip_pool.tile([P, KO, TQ], BF16, tag="wstrip16")
                nc.sync.dma_start(
                    w1_strip[:], w18.rearrange("(o p) n -> p o n", p=P)[:, :, half * TQ:(half + 1) * TQ])
                for m in range(4):
                    mm = half * 4 + m
                    psf = ps_w.tile([P, TQ], F32, tag="ps_w")
                    for ko in range(KO):
                        nc.tensor.matmul(
                            psf[:], w1_strip[:, ko, m * P:(m + 1) * P],
                            h2T8[:, ko, :],
                            start=(ko == 0), stop=(ko == KO - 1))
                    nc.scalar.activation(out=fT8[:, mm, :], in_=psf[:], func=AF.Gelu,
                                         bias=b1_pm[:, mm:mm + 1], scale=1.0)
            for half in range(2):
                w2_strip = wstrip_pool.tile([P, KO, TQ], BF16, tag="wstrip16")
                nc.sync.dma_start(
                    w2_strip[:], w28.rearrange("(o p) n -> p o n", p=P)[:, :, half * TQ:(half + 1) * TQ])
                for m in range(4):
                    mm = half * 4 + m
                    psz = ps_w.tile([P, TQ], F32, tag="ps_w")
                    for ko in range(KO):
                        nc.tensor.matmul(
                            psz[:], w2_strip[:, ko, m * P:(m + 1) * P],
                            fT8[:, ko, :],
                            start=(ko == 0), stop=(ko == KO - 1))
                    zbf = evac_pool.tile([P, TQ], BF16, tag="ybf")
                    nc.vector.tensor_scalar(
                        out=zbf[:], in0=psz[:], scalar1=b2_pm[:, mm:mm + 1],
                        scalar2=None, op0=ALU.add)
                    nc.vector.tensor_tensor(out=x1T[:, mm, :], in0=x1T[:, mm, :],
                                            in1=zbf[:], op=ALU.add)

            nc.sync.dma_start(out.rearrange("(o p) t -> p o t", p=P), x1T[:])

    nc.compile()
    return nc


_NC_CACHE = None


def _get_nc():
    global _NC_CACHE
    if _NC_CACHE is None:
        _NC_CACHE = build_kernel()
    return _NC_CACHE


def _prep_weights(Wq, Wk, Wv, Wo, W1, W2, ln1_g, ln1_b, ln2_g, ln2_b, b1):
    """Fold LN gammas into weights, scale x16 into fp8, and build the
    augmented-row constants (exact colsums of the QUANTIZED weights)."""
    wq = np.ascontiguousarray(np.transpose(Wq, (1, 0, 2)).reshape(D, D)) * ln1_g[:, None]
    wk = np.ascontiguousarray(np.transpose(Wk, (1, 0, 2)).reshape(D, D)) * ln1_g[:, None]
    wv = np.ascontiguousarray(np.transpose(Wv, (1, 0, 2)).reshape(D, D)) * ln1_g[:, None]
    w1 = W1 * ln2_g[:, None]

    def q8(w):
        return np.ascontiguousarray((w * SW)).astype(NP_F8)

    wq8, wk8, wv8 = q8(wq), q8(wk), q8(wv)
    wo8 = q8(Wo)
    w18 = np.ascontiguousarray(w1).astype(NP_BF)
    w28 = np.ascontiguousarray(W2).astype(NP_BF)

    def aug(w8, c):
        cs = w8.astype(np.float32).sum(axis=0)
        return np.ascontiguousarray(
            np.stack([-cs, SW * c]).astype(NP_BF))

    cq = ln1_b @ wq
    ck = ln1_b @ wk
    cv = ln1_b @ wv
    b1p = (b1 + ln2_b @ w1).astype(np.float32)
    return (wq8, wk8, wv8, wo8, w18, w28,
            aug(wq8, cq), aug(wk8, ck), aug(wv8, cv), b1p)


def make_in_maps(x, Wq, Wk, Wv, Wo, bo, ln1_g, ln1_b, ln2_g, ln2_b,
                 W1, b1, W2, b2):
    x = np.asarray(x, dtype=np.float32)
    (wq8, wk8, wv8, wo8, w18, w28, aq, ak, av, b1p) = _prep_weights(
        np.asarray(Wq, np.float32), np.asarray(Wk, np.float32),
        np.asarray(Wv, np.float32), np.asarray(Wo, np.float32),
        np.asarray(W1, np.float32), np.asarray(W2, np.float32),
        np.asarray(ln1_g, np.float32), np.asarray(ln1_b, np.float32),
        np.asarray(ln2_g, np.float32), np.asarray(ln2_b, np.float32),
        np.asarray(b1, np.float32))
    common = {
        "wq8": wq8, "wk8": wk8, "wv8": wv8, "wo8": wo8, "w18": w18, "w28": w28,
        "aug_q": aq, "aug_k": ak, "aug_v": av,
        "bo": np.asarray(bo, np.float32), "b1": b1p,
        "b2": np.asarray(b2, np.float32),
    }
    in_maps = []
    for core in range(8):
        b, c = divmod(core, 4)
        perm = np.concatenate([np.arange(c * TQ, (c + 1) * TQ),
                               np.arange(0, c * TQ), np.arange((c + 1) * TQ, T)])
        xb = x[b][perm]                       # [T, D], own tokens first
        xbT = np.ascontiguousarray(xb.T)      # [D, T]
        in_maps.append({
            "xrows": xb.astype(NP_BF),
            "xt8": xbT.astype(NP_F8),
            "xt0": np.ascontiguousarray(xbT[:, 0:TQ]).astype(np.float32),
            **common,
        })
    return in_maps


def kernel(x, Wq, Wk, Wv, Wo, bo, ln1_g, ln1_b, ln2_g, ln2_b, W1, b1, W2, b2,
           _trace=False):
    in_maps = make_in_maps(x, Wq, Wk, Wv, Wo, bo, ln1_g, ln1_b, ln2_g, ln2_b,
                           W1, b1, W2, b2)
    nc = _get_nc()
    res = run_bass_kernel_spmd(nc, in_maps, core_ids=list(range(8)), trace=_trace)
    out = np.empty((2, T, D), np.float32)
    for core in range(8):
        b, c = divmod(core, 4)
        out[b, c * TQ:(c + 1) * TQ] = res.results[core]["out"].T
    if _trace:
        kernel.last_results = res
    return out


# revision 14
# speedup vs baseline: 1.0263x; 1.0263x over previous
"""Trainium2 Bass kernel for nn_Block_49624052138029 (dense transformer block).

Strategy: data parallel over 8 NeuronCores; core i owns batch i//4 and query
chunk i%4 (512 tokens, permuted to the front host-side). K/V are computed
redundantly over all 2048 tokens of the core's batch (cheaper than on-chip
collectives on this stack).

v2 layout: everything feature-major ([feature on partitions, tokens on free])
with ZERO on-chip transposes in the hot path:
 - x arrives from host three ways: token-major bf16 (LayerNorm statistics
   only), feature-major fp8 (matmul moving operand), own-chunk feature-major
   f32 (residual accumulator).
 - LayerNorm 1 is applied *virtually*: the projections consume raw fp8 x^T
   plus one augmented contraction row (mov = mu*rstd, stat = -colsum(W)) that
   subtracts the mean inside PSUM, and the rstd scale is applied at PSUM
   evacuation (TT against an rstd broadcast built by a PE ones-matmul).
 - All dense matmuls (QKV/Wo/FFN) run fp8 DoubleRow (weights scaled x16 into
   fp8's normal range; compensated at evacuation). Contraction 1024 = 4 DR
   chunks instead of 8 bf16 chunks.
 - QK^T is row-tiled: head pair (2p, 2p+1) lives on partitions 0:64 / 64:128
   of feature chunk p, and the two 64-contraction matmuls run concurrently in
   the PE array via tile_position=(64,0) for the upper head.
 - Softmax: exp on ScalarE -> fp8, AV via fp8 DoubleRow with an appended
   ones-column so the denominator falls out of the same accumulation; the
   reciprocal 1/D = exp(-ln(D)) runs on ScalarE (DVE reciprocal on a
   single-partition row measured ~2.7us each).
 - LN2 statistics via PE ones-column matmuls on the feature-major residual
   (sum and sum-of-squares), row math on [1,512] tiles, normalized h2
   materialized fp8 for the FFN.
Output is written feature-major and transposed on the host.
"""

import os
import sys

for _p in ("/root/.axon_site", "/root/.axon_site/_ro/trn_rl_repo",
           "/root/.axon_site/_ro/pypackages", "/opt/trn_rl_repo", "/opt/pypackages"):
    if os.path.isdir(_p) and _p not in sys.path:
        sys.path.append(_p)

import numpy as np
import ml_dtypes

import concourse.bass as bass
import concourse.tile as tile
from concourse import bacc, mybir
from concourse.bass_utils import run_bass_kernel_spmd
from concourse.masks import make_identity

F32 = mybir.dt.float32
BF16 = mybir.dt.bfloat16
FP8 = mybir.dt.float8e4
AF = mybir.ActivationFunctionType
ALU = mybir.AluOpType
DR = mybir.MatmulPerfMode.DoubleRow

NP_F8 = ml_dtypes.float8_e4m3
NP_BF = ml_dtypes.bfloat16

D = 1024
H = 16
E = 64
T = 2048
TQ = 512
P = 128
KO = 8            # 128-feature chunks
KP = 4            # 256-feature DoubleRow chunks
SO = 16           # key chunks per batch
EPS = 1e-5
SW = 16.0         # fp8 weight scale
# scores carry x16 from q and x16 from k
SCALE_EXP = (D ** -0.5) / (SW * SW)


def build_kernel():
    nc = bacc.Bacc(None, target_bir_lowering=False, debug=False)

    xrows = nc.dram_tensor("xrows", [T, D], BF16, kind="ExternalInput")
    xt8 = nc.dram_tensor("xt8", [D, T], FP8, kind="ExternalInput")
    xt0 = nc.dram_tensor("xt0", [D, TQ], F32, kind="ExternalInput")
    wq8 = nc.dram_tensor("wq8", [D, D], FP8, kind="ExternalInput")
    wk8 = nc.dram_tensor("wk8", [D, D], FP8, kind="ExternalInput")
    wv8 = nc.dram_tensor("wv8", [D, D], FP8, kind="ExternalInput")
    wo8 = nc.dram_tensor("wo8", [D, D], FP8, kind="ExternalInput")
    w18 = nc.dram_tensor("w18", [D, D], BF16, kind="ExternalInput")
    w28 = nc.dram_tensor("w28", [D, D], BF16, kind="ExternalInput")
    aug_q = nc.dram_tensor("aug_q", [2, D], BF16, kind="ExternalInput")
    aug_k = nc.dram_tensor("aug_k", [2, D], BF16, kind="ExternalInput")
    aug_v = nc.dram_tensor("aug_v", [2, D], BF16, kind="ExternalInput")
    bo = nc.dram_tensor("bo", [D], F32, kind="ExternalInput")
    b1 = nc.dram_tensor("b1", [D], F32, kind="ExternalInput")
    b2 = nc.dram_tensor("b2", [D], F32, kind="ExternalInput")
    out = nc.dram_tensor("out", [D, TQ], F32, kind="ExternalOutput")
    scratch = nc.dram_tensor("scratch", [2 * 24 * P], BF16, kind="Internal")

    def pm(v):                      # [D] -> [128, 8] per-partition layout
        return v.rearrange("(o p) -> p o", p=P)

    def wdr(w):                     # [D, D] -> [p, a, t, n] DR stationary view
        return w.rearrange("(a t p) n -> p a t n", t=2, p=P)

    with tile.TileContext(nc) as tc:
        with (
            tc.tile_pool(name="singles", bufs=1) as singles,
            tc.tile_pool(name="persist", bufs=1) as persist,
            tc.tile_pool(name="xrow", bufs=3) as xrow_pool,
            tc.tile_pool(name="stats", bufs=6) as stats_pool,
            tc.tile_pool(name="lnrow", bufs=2) as lnrow_pool,
            tc.tile_pool(name="ln2row", bufs=1) as ln2row_pool,
            tc.tile_pool(name="evac", bufs=3) as evac_pool,
            tc.tile_pool(name="dbc", bufs=2) as dbc_pool,
            tc.tile_pool(name="wsmall", bufs=3) as wsmall_pool,
            tc.tile_pool(name="wstrip", bufs=2) as wstrip_pool,
            tc.tile_pool(name="exps", bufs=6) as exps_pool,
            tc.tile_pool(name="ps_w", bufs=2, space="PSUM") as ps_w,
            tc.tile_pool(name="ps_qk", bufs=2, space="PSUM") as ps_qk,
            tc.tile_pool(name="ps_u", bufs=2, space="PSUM") as ps_u,
        ):
            # ---------------- setup ----------------
            id_f32 = singles.tile([P, P], F32, name="id_f32")
            make_identity(nc, id_f32[:])
            ones64 = singles.tile([1, E], BF16, name="ones64")
            nc.vector.memset(ones64[:], 1.0)
            ones1x128 = singles.tile([1, P], BF16, name="ones1x128")
            nc.vector.memset(ones1x128[:], 1.0)
            onescol_f32 = singles.tile([P, 1], F32, name="onescol_f32")
            nc.vector.memset(onescol_f32[:], 1.0)
            eps_t = singles.tile([P, 1], F32, name="eps_t")
            nc.vector.memset(eps_t[:], EPS)

            bo_pm = singles.tile([P, KO], F32, name="bo_pm")
            nc.sync.dma_start(bo_pm[:], pm(bo))
            b1_pm = singles.tile([P, KO], F32, name="b1_pm")
            nc.sync.dma_start(b1_pm[:], pm(b1))
            b2_pm = singles.tile([P, KO], F32, name="b2_pm")
            nc.sync.dma_start(b2_pm[:], pm(b2))
            aq_sb = singles.tile([2, D], BF16, name="aq_sb")
            nc.sync.dma_start(aq_sb[:], aug_q[0:2, :])
            ak_sb = singles.tile([2, D], BF16, name="ak_sb")
            nc.sync.dma_start(ak_sb[:], aug_k[0:2, :])
            av_sb = singles.tile([2, D], BF16, name="av_sb")
            nc.sync.dma_start(av_sb[:], aug_v[0:2, :])

            # ---------------- persistent tiles ----------------
            xT8 = persist.tile([P, KO, T], FP8, name="xT8")        # 2 MB
            nc.sync.dma_start(xT8[:], xt8.rearrange("(o p) t -> p o t", p=P))
            sall = persist.tile([P, SO, 3], F32, name="sall")      # s, std, rstd
            rows_bf = persist.tile([2, T], BF16, name="rows_bf")
            rstd_row = persist.tile([1, T], BF16, name="rstd_row")
            rstd_bc = persist.tile([P, T], F32, name="rstd_bc")    # 1 MB
            kT8 = persist.tile([P, KO, T], BF16, name="kT8")       # 4 MB
            qT8 = persist.tile([P, KO, TQ], BF16, name="qT8")
            vP = persist.tile([P, SO // 2, 2, H, E + 1], FP8, name="vP")
            nc.vector.memset(vP[:, :, :, :, E], 1.0)
            oT8 = persist.tile([P, KO, TQ], FP8, name="oT8")
            x1T = persist.tile([P, KO, TQ], F32, name="x1T")       # 2 MB
            nc.sync.dma_start(x1T[:], xt0.rearrange("(o p) t -> p o t", p=P))
            h2T8 = persist.tile([P, KO, TQ], BF16, name="h2T8")
            fT8 = persist.tile([P, KO, TQ], BF16, name="fT8")

            # ---------------- LN1 statistics (two halves of 8 row-tiles) ----
            for half in range(2):
                for r in range(8):
                    rt = half * 8 + r
                    xr = xrow_pool.tile([P, D], BF16, tag="xr")
                    nc.sync.dma_start(xr[:], xrows[rt * P:(rt + 1) * P, :])
                    st = stats_pool.tile([P, 2, 6], F32, tag="bnstats")
                    xg = xr[:].rearrange("p (g d) -> p g d", g=2)
                    for g in range(2):
                        nc.vector.bn_stats(out=st[:, g, :], in_=xg[:, g, :])
                    mv = stats_pool.tile([P, 2], F32, tag="bnaggr")
                    nc.vector.bn_aggr(out=mv[:], in_=st[:])
                    # std = sqrt(var + eps); rstd = 1/std; s = mean*rstd
                    nc.scalar.activation(out=sall[:, rt, 1:2], in_=mv[:, 1:2],
                                         func=AF.Sqrt, bias=eps_t[:], scale=1.0)
                    nc.vector.reciprocal(out=sall[:, rt, 2:3],
                                         in_=sall[:, rt, 1:2])
                    nc.vector.tensor_tensor(
                        out=sall[:, rt, 0:1], in0=mv[:, 0:1],
                        in1=sall[:, rt, 2:3], op=ALU.mult)
                # transpose this half's stats into free-dim rows
                trp = ps_w.tile([P, TQ], F32, tag="ps_w")
                nc.tensor.transpose(
                    trp[0:24, 0:P],
                    sall[:, half * 8:(half + 1) * 8, :].rearrange(
                        "p a b -> p (a b)"),
                    id_f32[:])
                stg = xrow_pool.tile([24, P], BF16, tag="stg")
                nc.vector.tensor_copy(out=stg[:], in_=trp[0:24, 0:P])
                nc.sync.dma_start(
                    scratch.rearrange("(h q n) -> h q n", q=24, n=P)[half],
                    stg[:])
                scr = scratch.rearrange("(h a k n) -> h k a n", h=2, k=3, n=P)
                nc.sync.dma_start(
                    rows_bf[0:2, half * 1024:(half + 1) * 1024].rearrange(
                        "o (a n) -> o a n", n=P), scr[half, 0:2])
                nc.sync.dma_start(
                    rstd_row[0:1, half * 1024:(half + 1) * 1024].rearrange(
                        "o (a n) -> o a n", n=P), scr[half, 2:3])
                # rstd broadcast for this half via PE ones-matmul
                pb = ps_qk.tile([P, 2, TQ], F32, tag="ps_qk")
                for hh in range(2):
                    base = half * 1024 + hh * TQ
                    nc.tensor.matmul(pb[:, hh, :], ones1x128[:],
                                     rstd_row[0:1, base:base + TQ],
                                     start=True, stop=True)
                nc.vector.tensor_copy(
                    out=rstd_bc[:, half * 1024:(half + 1) * 1024],
                    in_=pb[:].rearrange("p a b -> p (a b)"))

            # ---------------- dense projection emitters ----------------
            def emit_q(he):
                wq_he = wsmall_pool.tile([P, KP, 2, P], FP8, tag="w_he")
                nc.sync.dma_start(wq_he[:], wdr(wq8)[:, :, :, he * P:(he + 1) * P])
                psq = ps_w.tile([P, TQ], F32, tag="ps_w")
                for a in range(KP):
                    nc.tensor.matmul(psq[:], wq_he[:, a], xT8[:, 2 * a:2 * a + 2, 0:TQ],
                                     start=(a == 0), stop=False, perf_mode=DR)
                nc.tensor.matmul(psq[:], aq_sb[:, he * P:(he + 1) * P],
                                 rows_bf[0:2, 0:TQ], start=False, stop=True)
                nc.vector.tensor_tensor(out=qT8[:, he, :], in0=psq[:],
                                        in1=rstd_bc[:, 0:TQ], op=ALU.mult)

            def emit_k_group(he, t, wk_he):
                tsl = slice(t * TQ, (t + 1) * TQ)
                psk = ps_w.tile([P, TQ], F32, tag="ps_w")
                for a in range(KP):
                    nc.tensor.matmul(psk[:], wk_he[:, a], xT8[:, 2 * a:2 * a + 2, tsl],
                                     start=(a == 0), stop=False, perf_mode=DR)
                nc.tensor.matmul(psk[:], ak_sb[:, he * P:(he + 1) * P],
                                 rows_bf[0:2, tsl], start=False, stop=True)
                nc.vector.tensor_tensor(out=kT8[:, he, tsl], in0=psk[:],
                                        in1=rstd_bc[:, tsl], op=ALU.mult)

            def emit_k(he, t_list=None):
                wk_he = wsmall_pool.tile([P, KP, 2, P], FP8, tag="w_he")
                nc.sync.dma_start(wk_he[:], wdr(wk8)[:, :, :, he * P:(he + 1) * P])
                for t in (range(4) if t_list is None else t_list):
                    emit_k_group(he, t, wk_he)
                return wk_he

            def emit_v_load(nh):
                wv_strip = wstrip_pool.tile([P, KP, 2, TQ], FP8, tag="wstrip")
                nc.sync.dma_start(
                    wv_strip[:], wdr(wv8)[:, :, :, nh * TQ:(nh + 1) * TQ])
                return wv_strip

            def emit_v_group(nh, so, wv_strip):
                ssl = slice(so * P, (so + 1) * P)
                psv = ps_w.tile([P, TQ], F32, tag="ps_w")
                for a in range(KP):
                    nc.tensor.matmul(psv[:], xT8[:, 2 * a:2 * a + 2, ssl],
                                     wv_strip[:, a], start=(a == 0),
                                     stop=False, perf_mode=DR)
                nc.tensor.matmul(psv[:], rows_bf[0:2, ssl],
                                 av_sb[:, nh * TQ:(nh + 1) * TQ],
                                 start=False, stop=True)
                nc.vector.tensor_scalar(
                    out=vP[:, so // 2, so % 2, nh * 8:(nh + 1) * 8, 0:E],
                    in0=psv[:].rearrange("p (h e) -> p h e", e=E),
                    scalar1=sall[:, so, 2:3], scalar2=None, op0=ALU.mult)

            def emit_v(nh, wv_strip=None, so_list=None):
                if wv_strip is None:
                    wv_strip = emit_v_load(nh)
                for so in (range(SO) if so_list is None else so_list):
                    emit_v_group(nh, so, wv_strip)
                return wv_strip

            # ---------------- attention ----------------
            def emit_attn(pair, dense_queue):
                ha, hb = 2 * pair, 2 * pair + 1
                psuA = ps_u.tile([P, TQ], F32, tag="ps_u", name="psuA")
                psuB = ps_u.tile([P, TQ], F32, tag="ps_u", name="psuB")
                for sp in range(SO // 2):
                    if dense_queue:
                        dense_queue.pop(0)()
                    pssA = ps_qk.tile([P, 2, TQ], F32, tag="ps_qk", name="pssA")
                    pssB = ps_qk.tile([P, 2, TQ], F32, tag="ps_qk", name="pssB")
                    for j in range(2):
                        so = 2 * sp + j
                        ssl = slice(so * P, (so + 1) * P)
                        nc.tensor.matmul(pssA[:, j, :], kT8[0:E, pair, ssl],
                                         qT8[0:E, pair, :], start=True, stop=True)
                        nc.tensor.matmul(pssB[:, j, :], kT8[E:P, pair, ssl],
                                         qT8[E:P, pair, :], start=True, stop=True,
                                         tile_position=(E, 0))
                    esA = exps_pool.tile([P, 2, TQ], FP8, tag="exps", name="esA")
                    nc.scalar.activation(out=esA[:], in_=pssA[:], func=AF.Exp,
                                         scale=SCALE_EXP)
                    esB = exps_pool.tile([P, 2, TQ], FP8, tag="exps", name="esB")
                    nc.scalar.activation(out=esB[:], in_=pssB[:], func=AF.Exp,
                                         scale=SCALE_EXP)
                    nc.tensor.matmul(psuA[0:E + 1, :], vP[:, sp, :, ha, :], esA[:],
                                     start=(sp == 0), stop=(sp == SO // 2 - 1),
                                     perf_mode=DR)
                    nc.tensor.matmul(psuB[0:E + 1, :], vP[:, sp, :, hb, :], esB[:],
                                     start=(sp == 0), stop=(sp == SO // 2 - 1),
                                     perf_mode=DR)
                # 1/D on DVE (keeps ScalarE's Exp table hot)
                dinvs = []
                for psu in (psuA, psuB):
                    dinv_f = lnrow_pool.tile([1, TQ], F32, tag="lnd")
                    nc.vector.reciprocal(out=dinv_f[:], in_=psu[E:E + 1, :])
                    dinv = lnrow_pool.tile([1, TQ], BF16, tag="dinv")
                    nc.vector.tensor_copy(out=dinv[:], in_=dinv_f[:])
                    dinvs.append(dinv)

                def epi():
                    for side, (psu, dinv) in enumerate(zip((psuA, psuB), dinvs)):
                        psb = ps_w.tile([P, TQ], F32, tag="ps_w")
                        nc.tensor.matmul(psb[0:E, :], ones64[:], dinv[:],
                                         start=True, stop=True)
                        dbc = dbc_pool.tile([E, TQ], BF16, tag="dbc")
                        nc.vector.tensor_copy(out=dbc[:], in_=psb[0:E, :])
                        nc.vector.tensor_tensor(
                            out=oT8[side * E:(side + 1) * E, pair, :],
                            in0=psu[0:E, :], in1=dbc[:], op=ALU.mult)
                return epi

            for he in range(KO):
                emit_q(he)
            emit_k(0)
            emit_v(0)
            # stage remaining dense work (K chunks 1..7, V half 1) as
            # per-group closures popped one per sp inside the attention loop
            # so the PE never idles long enough for HAM to re-throttle.
            dense_queue = []
            k_strips = {}
            for he in range(1, KO):
                wk_he = wsmall_pool.tile([P, KP, 2, P], FP8, tag="w_he2",
                                         name=f"wk{he}")
                nc.sync.dma_start(wk_he[:], wdr(wk8)[:, :, :, he * P:(he + 1) * P])
                k_strips[he] = wk_he
            v1_strip = emit_v_load(1)
            for he in range(1, KO):
                for t in range(4):
                    dense_queue.append(
                        lambda he=he, t=t: emit_k_group(he, t, k_strips[he]))
            for so in range(SO):
                dense_queue.append(
                    lambda so=so: emit_v_group(1, so, v1_strip))
            for pair in range(KO):
                epi = emit_attn(pair, dense_queue)
                if dense_queue:
                    dense_queue.pop(0)()
                epi()
            assert not dense_queue

            # ---------------- Wo projection + residual ----------------
            for half in range(2):
                wo_strip = wstrip_pool.tile([P, KP, 2, TQ], FP8, tag="wstrip")
                nc.sync.dma_start(
                    wo_strip[:], wdr(wo8)[:, :, :, half * TQ:(half + 1) * TQ])
                for m in range(4):
                    mm = half * 4 + m
                    psy = ps_w.tile([P, TQ], F32, tag="ps_w")
                    for a in range(KP):
                        nc.tensor.matmul(
                            psy[:], wo_strip[:, a, :, m * P:(m + 1) * P],
                            oT8[:, 2 * a:2 * a + 2, :],
                            start=(a == 0), stop=(a == KP - 1), perf_mode=DR)
                    ybf = evac_pool.tile([P, TQ], BF16, tag="ybf")
                    nc.vector.tensor_scalar(
                        out=ybf[:], in0=psy[:], scalar1=1.0 / (SW * SW),
                        scalar2=bo_pm[:, mm:mm + 1], op0=ALU.mult, op1=ALU.add)
                    nc.vector.tensor_tensor(out=x1T[:, mm, :], in0=x1T[:, mm, :],
                                            in1=ybf[:], op=ALU.add)

            # ---------------- LN2 (feature-major, PE column sums) ----------
            psS = ps_w.tile([P, TQ], F32, tag="ps_w")
            for ko in range(KO):
                nc.tensor.matmul(psS[0:1, :], onescol_f32[:], x1T[:, ko, :],
                                 start=(ko == 0), stop=(ko == KO - 1))
            psQ2 = ps_w.tile([P, TQ], F32, tag="ps_w")
            for ko in range(KO):
                sq = evac_pool.tile([P, TQ], F32, tag="sq")
                nc.vector.tensor_tensor(out=sq[:], in0=x1T[:, ko, :],
                                        in1=x1T[:, ko, :], op=ALU.mult)
                nc.tensor.matmul(psQ2[0:1, :], onescol_f32[:], sq[:],
                                 start=(ko == 0), stop=(ko == KO - 1))
            mu2 = ln2row_pool.tile([1, TQ], F32, tag="mu2")
            nc.vector.tensor_scalar(out=mu2[:], in0=psS[0:1, :], scalar1=1.0 / D,
                                    scalar2=None, op0=ALU.mult)
            m2 = ln2row_pool.tile([1, TQ], F32, tag="m2")
            nc.vector.tensor_scalar(out=m2[:], in0=psQ2[0:1, :], scalar1=1.0 / D,
                                    scalar2=None, op0=ALU.mult)
            var2 = ln2row_pool.tile([1, TQ], F32, tag="var2")
            nc.vector.tensor_tensor(out=var2[:], in0=mu2[:], in1=mu2[:],
                                    op=ALU.mult)
            nc.vector.tensor_tensor(out=var2[:], in0=m2[:], in1=var2[:],
                                    op=ALU.subtract)
            lnv = ln2row_pool.tile([1, TQ], F32, tag="lnv")
            nc.scalar.activation(out=lnv[:], in_=var2[:], func=AF.Ln,
                                 bias=eps_t[0:1, :])
            rstd2 = ln2row_pool.tile([1, TQ], BF16, tag="rstd2")
            nc.scalar.activation(out=rstd2[:], in_=lnv[:], func=AF.Exp, scale=-0.5)
            rstd2f = ln2row_pool.tile([1, TQ], F32, tag="rstd2f")
            nc.vector.tensor_copy(out=rstd2f[:], in_=rstd2[:])
            s2 = ln2row_pool.tile([1, TQ], BF16, tag="s2")
            nc.vector.tensor_tensor(out=s2[:], in0=mu2[:], in1=rstd2f[:],
                                    op=ALU.mult)
            psR = ps_w.tile([P, TQ], F32, tag="ps_w")
            nc.tensor.matmul(psR[:], ones1x128[:], rstd2[:], start=True, stop=True)
            psM = ps_w.tile([P, TQ], F32, tag="ps_w")
            nc.tensor.matmul(psM[:], ones1x128[:], s2[:], start=True, stop=True)
            for ko in range(KO):
                t1 = evac_pool.tile([P, TQ], BF16, tag="t1")
                nc.vector.tensor_tensor(out=t1[:], in0=x1T[:, ko, :], in1=psR[:],
                                        op=ALU.mult)
                nc.vector.tensor_tensor(out=h2T8[:, ko, :], in0=t1[:], in1=psM[:],
                                        op=ALU.subtract)

            # ---------------- FFN ----------------
            for half in range(2):
                w1_strip = wstrip_pool.tile([P, KO, TQ], BF16, tag="wstrip16")
                nc.sync.dma_start(
                    w1_strip[:], w18.rearrange("(o p) n -> p o n", p=P)[:, :, half * TQ:(half + 1) * TQ])
                for m in range(4):
                    mm = half * 4 + m
                    psf = ps_w.tile([P, TQ], F32, tag="ps_w")
                    for ko in range(KO):
                        nc.tensor.matmul(
                            psf[:], w1_strip[:, ko, m * P:(m + 1) * P],
                            h2T8[:, ko, :],
                            start=(ko == 0), stop=(ko == KO - 1))
                    nc.scalar.activation(out=fT8[:, mm, :], in_=psf[:], func=AF.Gelu,
                                         bias=b1_pm[:, mm:mm + 1], scale=1.0)
            for half in range(2):
                w2_strip = wstrip_pool.tile([P, KO, TQ], BF16, tag="wstrip16")
                nc.sync.dma_start(
                    w2_strip[:], w28.rearrange("(o p) n -> p o n", p=P)[:, :, half * TQ:(half + 1) * TQ])
                for m in range(4):
                    mm = half * 4 + m
                    psz = ps_w.tile([P, TQ], F32, tag="ps_w")
                    for ko in range(KO):
                        nc.tensor.matmul(
                            psz[:], w2_strip[:, ko, m * P:(m + 1) * P],
                            fT8[:, ko, :],
                            start=(ko == 0), stop=(ko == KO - 1))
                    zbf = evac_pool.tile([P, TQ], BF16, tag="ybf")
                    nc.vector.tensor_scalar(
                        out=zbf[:], in0=psz[:], scalar1=b2_pm[:, mm:mm + 1],
                        scalar2=None, op0=ALU.add)
                    nc.vector.tensor_tensor(out=x1T[:, mm, :], in0=x1T[:, mm, :],
                                            in1=zbf[:], op=ALU.add)

            nc.sync.dma_start(out.rearrange("(o p) t -> p o t", p=P), x1T[:])

    nc.compile()
    return nc


_NC_CACHE = None


def _get_nc():
    global _NC_CACHE
    if _NC_CACHE is None:
        _NC_CACHE = build_kernel()
    return _NC_CACHE


def _prep_weights(Wq, Wk, Wv, Wo, W1, W2, ln1_g, ln1_b, ln2_g, ln2_b, b1):
    """Fold LN gammas into weights, scale x16 into fp8, and build the
    augmented-row constants (exact colsums of the QUANTIZED weights)."""
    wq = np.ascontiguousarray(np.transpose(Wq, (1, 0, 2)).reshape(D, D)) * ln1_g[:, None]
    wk = np.ascontiguousarray(np.transpose(Wk, (1, 0, 2)).reshape(D, D)) * ln1_g[:, None]
    wv = np.ascontiguousarray(np.transpose(Wv, (1, 0, 2)).reshape(D, D)) * ln1_g[:, None]
    w1 = W1 * ln2_g[:, None]

    def q8(w):
        return np.ascontiguousarray((w * SW)).astype(NP_F8)

    wq8, wk8, wv8 = q8(wq), q8(wk), q8(wv)
    wo8 = q8(Wo)
    w18 = np.ascontiguousarray(w1).astype(NP_BF)
    w28 = np.ascontiguousarray(W2).astype(NP_BF)

    def aug(w8, c):
        cs = w8.astype(np.float32).sum(axis=0)
        return np.ascontiguousarray(
            np.stack([-cs, SW * c]).astype(NP_BF))

    cq = ln1_b @ wq
    ck = ln1_b @ wk
    cv = ln1_b @ wv
    b1p = (b1 + ln2_b @ w1).astype(np.float32)
    return (wq8, wk8, wv8, wo8, w18, w28,
            aug(wq8, cq), aug(wk8, ck), aug(wv8, cv), b1p)


def make_in_maps(x, Wq, Wk, Wv, Wo, bo, ln1_g, ln1_b, ln2_g, ln2_b,
                 W1, b1, W2, b2):
    x = np.asarray(x, dtype=np.float32)
    (wq8, wk8, wv8, wo8, w18, w28, aq, ak, av, b1p) = _prep_weights(
        np.asarray(Wq, np.float32), np.asarray(Wk, np.float32),
        np.asarray(Wv, np.float32), np.asarray(Wo, np.float32),
        np.asarray(W1, np.float32), np.asarray(W2, np.float32),
        np.asarray(ln1_g, np.float32), np.asarray(ln1_b, np.float32),
        np.asarray(ln2_g, np.float32), np.asarray(ln2_b, np.float32),
        np.asarray(b1, np.float32))
    common = {
        "wq8": wq8, "wk8": wk8, "wv8": wv8, "wo8": wo8, "w18": w18, "w28": w28,
        "aug_q": aq, "aug_k": ak, "aug_v": av,
        "bo": np.asarray(bo, np.float32), "b1": b1p,
        "b2": np.asarray(b2, np.float32),
    }
    in_maps = []
    for core in range(8):
        b, c = divmod(core, 4)
        perm = np.concatenate([np.arange(c * TQ, (c + 1) * TQ),
                               np.arange(0, c * TQ), np.arange((c + 1) * TQ, T)])
        xb = x[b][perm]                       # [T, D], own tokens first
        xbT = np.ascontiguousarray(xb.T)      # [D, T]
        in_maps.append({
            "xrows": xb.astype(NP_BF),
            "xt8": xbT.astype(NP_F8),
            "xt0": np.ascontiguousarray(xbT[:, 0:TQ]).astype(np.float32),
            **common,
        })
    return in_maps


def kernel(x, Wq, Wk, Wv, Wo, bo, ln1_g, ln1_b, ln2_g, ln2_b, W1, b1, W2, b2,
           _trace=False):
    in_maps = make_in_maps(x, Wq, Wk, Wv, Wo, bo, ln1_g, ln1_b, ln2_g, ln2_b,
                           W1, b1, W2, b2)
    nc = _get_nc()
    res = run_bass_kernel_spmd(nc, in_maps, core_ids=list(range(8)), trace=_trace)
    out = np.empty((2, T, D), np.float32)
    for core in range(8):
        b, c = divmod(core, 4)
        out[b, c * TQ:(c + 1) * TQ] = res.results[core]["out"].T
    if _trace:
        kernel.last_results = res
    return out


# revision 15
# speedup vs baseline: 1.2104x; 1.1794x over previous
"""Trainium2 Bass kernel for nn_Block_49624052138029 (dense transformer block).

Strategy: pure data parallelism across 8 NeuronCores. Core i handles batch
b = i//4 and query-chunk c = i%4 (512 of the 2048 tokens). The host permutes
each core's batch slice so its own 512 query rows come first; attention is
permutation-invariant over keys, so K/V row order doesn't matter. Each core
redundantly computes LN1 + K + V over all 2048 rows of its batch (cheaper
than on-chip collectives on this stack), and Q/attention/Wo/FFN only for its
own 512 rows.

On-chip layout: activations live feature-major ([D on partitions, tokens on
free]) for matmuls; LayerNorm runs row-major and the gamma/beta application is
fused into the PSUM-evacuation of the PE transpose (where D sits on
partitions). Attention computes S^T = K_h^T-chunks.T @ Q_h^T per head with an
exp() evacuation on ScalarE, and AV appends an all-ones column to V so the
softmax denominator falls out of the same accumulation (row 64 of U^T).

All matmul operands are bf16 (f32 PSUM accumulation); LN statistics,
residual stream and softmax denominators stay f32.
"""

import os
import sys

for _p in ("/root/.axon_site", "/root/.axon_site/_ro/trn_rl_repo",
           "/root/.axon_site/_ro/pypackages", "/opt/trn_rl_repo", "/opt/pypackages"):
    if os.path.isdir(_p) and _p not in sys.path:
        sys.path.append(_p)

import numpy as np
import ml_dtypes

import concourse.bass as bass
import concourse.tile as tile
from concourse import bacc, mybir
from concourse.bass_utils import run_bass_kernel_spmd
from concourse.masks import make_identity

F32 = mybir.dt.float32
BF16 = mybir.dt.bfloat16
FP8 = mybir.dt.float8e4
AF = mybir.ActivationFunctionType
ALU = mybir.AluOpType
AX = mybir.AxisListType

D = 1024          # model dim
H = 16            # heads
E = 64            # head dim
T = 2048          # tokens per batch
TQ = 512          # tokens owned by this core
P = 128
KO = D // P       # 8 feature chunks
RT_ALL = T // P   # 16 row tiles per batch
RT_OWN = TQ // P  # 4 row tiles owned
SO = T // P       # 16 key chunks
EPS = 1e-5
SCALE = 1.0 / 32.0  # D ** -0.5


def _layer_norm_rows(nc, sng, xrow_ap, stats_pool, y_out_ap, y_eng="dve"):
    """Row-major LayerNorm core: y = (x - mean(x)) * rsqrt(var(x) + eps).

    xrow_ap: [128, 1024] f32 SBUF; y_out_ap: [128, 1024] (any dtype) SBUF.
    gamma/beta are NOT applied here (folded into the weights host-side).
    y_eng picks the engine for the [128,1024] normalize op so callers can
    balance DVE vs ACT load per phase.
    """
    stats = stats_pool.tile([P, 2, 6], F32, tag="bnstats")
    xg = xrow_ap.rearrange("p (g d) -> p g d", g=2)
    for g in range(2):
        nc.vector.bn_stats(out=stats[:, g, :], in_=xg[:, g, :])
    mv = stats_pool.tile([P, 2], F32, tag="bnaggr")
    nc.vector.bn_aggr(out=mv[:], in_=stats[:])
    rstd = stats_pool.tile([P, 1], F32, tag="rstd")
    # rstd = 1 / sqrt(var + eps)
    nc.scalar.activation(out=rstd[:], in_=mv[:, 1:2], func=AF.Sqrt,
                         bias=sng["eps"][:], scale=1.0)
    nc.vector.reciprocal(out=rstd[:], in_=rstd[:])
    if y_eng == "act":
        # y = x*rstd + (-mean*rstd) on ScalarE
        nmr = stats_pool.tile([P, 1], F32, tag="nmr")
        nc.vector.tensor_scalar(
            out=nmr[:], in0=mv[:, 0:1], scalar1=rstd[:], scalar2=-1.0,
            op0=ALU.mult, op1=ALU.mult)
        nc.scalar.activation(out=y_out_ap, in_=xrow_ap, func=AF.Identity,
                             scale=rstd[:], bias=nmr[:])
    else:
        nc.vector.tensor_scalar(
            out=y_out_ap, in0=xrow_ap, scalar1=mv[:, 0:1], scalar2=rstd[:],
            op0=ALU.subtract, op1=ALU.mult)


def build_kernel():
    nc = bacc.Bacc(None, target_bir_lowering=False, debug=False)

    xb = nc.dram_tensor("xb", [T, D], F32, kind="ExternalInput")
    wq = nc.dram_tensor("wq", [D, D], BF16, kind="ExternalInput")
    wk = nc.dram_tensor("wk", [D, D], BF16, kind="ExternalInput")
    wv = nc.dram_tensor("wv", [D, D], BF16, kind="ExternalInput")
    wo = nc.dram_tensor("wo", [D, D], BF16, kind="ExternalInput")
    w1 = nc.dram_tensor("w1", [D, D], BF16, kind="ExternalInput")
    w2 = nc.dram_tensor("w2", [D, D], BF16, kind="ExternalInput")
    cq = nc.dram_tensor("cq", [D], F32, kind="ExternalInput")
    ck = nc.dram_tensor("ck", [D], F32, kind="ExternalInput")
    bo = nc.dram_tensor("bo", [D], F32, kind="ExternalInput")
    b1 = nc.dram_tensor("b1", [D], F32, kind="ExternalInput")
    b2 = nc.dram_tensor("b2", [D], F32, kind="ExternalInput")
    out = nc.dram_tensor("out", [TQ, D], F32, kind="ExternalOutput")

    # per-feature params as [128, 8] (partition p, chunk o) for feature-major use
    def pm(dram_vec):
        return dram_vec.rearrange("(o p) -> p o", p=P)

    with tile.TileContext(nc) as tc:
        with (
            tc.tile_pool(name="singles", bufs=1) as singles,
            tc.tile_pool(name="persist", bufs=1) as persist,
            tc.tile_pool(name="hrow", bufs=4) as hrow_pool,
            tc.tile_pool(name="stats", bufs=6) as stats_pool,
            tc.tile_pool(name="wstrip", bufs=2) as wstrip_pool,
        ):
            # ---------------- setup ----------------
            sng = {}
            id_bf = singles.tile([P, P], BF16, name="id_bf")
            make_identity(nc, id_bf[:])
            id_f32 = singles.tile([P, P], F32, name="id_f32")
            make_identity(nc, id_f32[:])
            sng["eps"] = singles.tile([P, 1], F32, name="eps")
            nc.vector.memset(sng["eps"][:], EPS)
            ones64 = singles.tile([1, E], BF16, name="ones64")
            nc.vector.memset(ones64[:], 1.0)


            cq_pm = singles.tile([P, KO], F32, name="cq_pm")
            nc.sync.dma_start(cq_pm[:], pm(cq))
            ck_pm = singles.tile([P, KO], F32, name="ck_pm")
            nc.sync.dma_start(ck_pm[:], pm(ck))
            bo_pm = singles.tile([P, KO], F32, name="bo_pm")
            nc.sync.dma_start(bo_pm[:], pm(bo))
            bf1_pm = singles.tile([P, KO], F32, name="bf1_pm")
            nc.sync.dma_start(bf1_pm[:], pm(b1))
            bf2_pm = singles.tile([P, KO], F32, name="bf2_pm")
            nc.sync.dma_start(bf2_pm[:], pm(b2))

            # ---------------- persistent activations ----------------
            kT = persist.tile([P, KO, T], BF16, name="kT")          # 4 MB
            vP = persist.tile([P, SO // 2, 2, H, E + 1], FP8, name="vP")  # 2.08 MB
            qPack = persist.tile([P, KO, 2, TQ], BF16, name="qPack")  # 2 MB
            nc.vector.memset(qPack[:], 0.0)
            oT = persist.tile([P, KO, TQ], BF16, name="oT")         # 1 MB
            x1 = persist.tile([P, RT_OWN, D], F32, name="x1")       # 2 MB
            h2T = persist.tile([P, KO, TQ], BF16, name="h2T")       # 1 MB
            fT = persist.tile([P, KO, TQ], BF16, name="fT")         # 1 MB

            # ones column of vP (softmax denominator trick)
            nc.vector.memset(vP[:, :, :, :, E], 1.0)

            # ---------------- phase 1: LN1 over all rows -> hT ----------------
            ps_w_ctx = tc.tile_pool(name="ps_w", bufs=2, space="PSUM")
            ps_w = ps_w_ctx.__enter__()
            ps_tr_ctx = tc.tile_pool(name="ps_tr", bufs=2, space="PSUM")
            ps_tr = ps_tr_ctx.__enter__()
            ctx12 = tc.tile_pool(name="hTp", bufs=1)
            hT_pool = ctx12.__enter__()
            wsmall_ctx = tc.tile_pool(name="wsmall", bufs=3)
            wsmall_pool = wsmall_ctx.__enter__()
            xrow_ctx = tc.tile_pool(name="xrow", bufs=3)
            xrow_pool = xrow_ctx.__enter__()
            hT = hT_pool.tile([P, KO, T], BF16, name="hT")          # 4 MB
            for rt in range(RT_ALL):
                if rt < RT_OWN:
                    x_t = x1[:, rt, :]   # own rows: keep the raw x for residual
                    nc.sync.dma_start(x_t, xb[rt * P:(rt + 1) * P, :])
                else:
                    x_tile = xrow_pool.tile([P, D], F32, tag="xrow")
                    nc.sync.dma_start(x_tile[:], xb[rt * P:(rt + 1) * P, :])
                    x_t = x_tile[:]
                y_row = hrow_pool.tile([P, D], BF16, tag="hrow")
                _layer_norm_rows(nc, sng, x_t, stats_pool, y_row[:])
                # gamma/beta are folded into the weights host-side, so the
                # transpose evacuation is a plain copy (batched 2 chunks/op)
                trp = ps_tr.tile([P, KO, P], BF16, tag="tr")
                for ko in range(KO):
                    nc.tensor.transpose(trp[:, ko, :], y_row[:, ko * P:(ko + 1) * P], id_bf[:])
                nc.scalar.copy(out=hT[:, :, rt * P:(rt + 1) * P], in_=trp[:])

            # ---------------- phases 2+3: q/k/v interleaved with attention ----------------
            xrow_ctx.__exit__(None, None, None)
            ps_tr_ctx.__exit__(None, None, None)
            exps_ctx = tc.tile_pool(name="exps", bufs=8)
            exps_pool = exps_ctx.__enter__()
            evac_ctx = tc.tile_pool(name="evac", bufs=4)
            evac_pool = evac_ctx.__enter__()
            ps_qk_ctx = tc.tile_pool(name="ps_qk", bufs=2, space="PSUM")
            ps_qk = ps_qk_ctx.__enter__()
            ps_u_ctx = tc.tile_pool(name="ps_u", bufs=2, space="PSUM")
            ps_u = ps_u_ctx.__enter__()

            # Dense k/v matmuls are emitted right after each attention head
            # pair (lower scheduler priority), so the in-order PE fills
            # exp-latency gaps with dense work instead of idling.
            def emit_k(he):
                wk_he = wsmall_pool.tile([P, KO, P], BF16, tag="w_he", name="wk_he")
                nc.sync.dma_start(
                    wk_he[:], wk.rearrange("(o p) n -> p o n", p=P)[:, :, he * P:(he + 1) * P])
                for t in range(RT_ALL // 4):
                    psk = ps_w.tile([P, 512], F32, tag="ps_w", name="psk")
                    for ko in range(KO):
                        nc.tensor.matmul(
                            psk[:], wk_he[:, ko, :], hT[:, ko, t * 512:(t + 1) * 512],
                            start=(ko == 0), stop=(ko == KO - 1))
                    nc.vector.tensor_scalar_add(
                        out=kT[:, he, t * 512:(t + 1) * 512], in0=psk[:],
                        scalar1=ck_pm[:, he:he + 1])

            def emit_q(he):
                wq_he = wsmall_pool.tile([P, KO, P], BF16, tag="w_he", name="wq_he")
                nc.sync.dma_start(
                    wq_he[:], wq.rearrange("(o p) n -> p o n", p=P)[:, :, he * P:(he + 1) * P])
                psq = ps_w.tile([P, 512], F32, tag="ps_w", name="psq")
                for ko in range(KO):
                    nc.tensor.matmul(
                        psq[:], wq_he[:, ko, :], hT[:, ko, 0:TQ],
                        start=(ko == 0), stop=(ko == KO - 1))
                nc.vector.tensor_scalar_add(
                    out=qPack[0:E, he, 0, :], in0=psq[0:E, :],
                    scalar1=cq_pm[0:E, he:he + 1])
                nc.vector.tensor_scalar_add(
                    out=qPack[E:P, he, 1, :], in0=psq[E:P, :],
                    scalar1=cq_pm[E:P, he:he + 1])

            def emit_v_load(nh):
                wv_strip = wstrip_pool.tile([P, KO, 512], BF16, tag="wstrip", name="wv_strip")
                nc.sync.dma_start(
                    wv_strip[:], wv.rearrange("(o p) n -> p o n", p=P)[:, :, nh * 512:(nh + 1) * 512])
                return wv_strip

            def emit_v(nh, wv_strip=None, so_list=None, evac_eng=None):
                if wv_strip is None:
                    wv_strip = emit_v_load(nh)
                for so in (range(SO) if so_list is None else so_list):
                    psv = ps_w.tile([P, 512], F32, tag="ps_w", name="psv")
                    for ko in range(KO):
                        nc.tensor.matmul(
                            psv[:], hT[:, ko, so * P:(so + 1) * P], wv_strip[:, ko, :],
                            start=(ko == 0), stop=(ko == KO - 1))
                    if evac_eng == "scalar":
                        nc.scalar.copy(
                            out=vP[:, so // 2, so % 2, nh * 8:(nh + 1) * 8, 0:E],
                            in_=psv[:].rearrange("p (h e) -> p h e", e=E))
                    else:
                        nc.vector.tensor_copy(
                            out=vP[:, so // 2, so % 2, nh * 8:(nh + 1) * 8, 0:E],
                            in_=psv[:].rearrange("p (h e) -> p h e", e=E))

            def emit_attn(h, pend_epi=None):
                # Returns this head's deferred epilogue closure. The caller
                # passes the PREVIOUS head's closure, flushed mid-way through
                # this head's QK stream so the in-order PE never waits on the
                # DVE reciprocal chain (was a ~2.5us stall per head).
                pbase = (h % 2) * E
                ko_h = h // 2
                psu = ps_u.tile([P, 512], F32, tag="ps_u", name="psu")
                for sp in range(SO // 2):
                    if sp == 3 and pend_epi is not None:
                        pend_epi()
                        pend_epi = None
                    pss = ps_qk.tile([P, 2, 512], F32, tag="ps_qk", name="pss")
                    for j in range(2):
                        so = 2 * sp + j
                        # full-K stationary (FWL-eligible); the other head's
                        # rows meet zeros in the packed q, so the sum is exact
                        nc.tensor.matmul(
                            pss[:, j, :],
                            kT[:, ko_h, so * P:(so + 1) * P],
                            qPack[:, ko_h, h % 2, :],
                            start=True, stop=True)
                    es = exps_pool.tile([P, 2, 512], FP8, tag="exps", name="es")
                    nc.scalar.activation(out=es[:], in_=pss[:], func=AF.Exp, scale=SCALE)
                    # fp8 DoubleRow: virtual K=256 sums both key chunks at once
                    nc.tensor.matmul(
                        psu[0:E + 1, :], vP[:, sp, :, h, :], es[:],
                        start=(sp == 0), stop=(sp == SO // 2 - 1),
                        perf_mode=mybir.MatmulPerfMode.DoubleRow)
                dinv_f = stats_pool.tile([1, TQ], F32, tag="dinv_f", name="dinv_f")
                nc.vector.reciprocal(out=dinv_f[:], in_=psu[E:E + 1, :])
                dinv = stats_pool.tile([1, TQ], BF16, tag="dinv", name="dinv")
                nc.vector.tensor_copy(out=dinv[:], in_=dinv_f[:])

                def epi():
                    psb = ps_qk.tile([P, 2, 512], F32, tag="ps_qk", name="psb")
                    nc.tensor.matmul(psb[0:E, 0, :], ones64[:], dinv[:],
                                     start=True, stop=True)
                    dbc = evac_pool.tile([E, 512], BF16, tag="dbc", name="dbc")
                    nc.vector.tensor_copy(out=dbc[:], in_=psb[0:E, 0, :])
                    nc.vector.tensor_tensor(
                        out=oT[pbase:pbase + E, ko_h, :], in0=psu[0:E, :],
                        in1=dbc[:], op=ALU.mult)
                return epi

            for he in range(KO):
                emit_q(he)
            emit_k(0)
            emit_v(0)
            pend = None
            for pair in range(KO):
                pend = emit_attn(2 * pair, pend)
                if pair + 1 < KO:
                    emit_k(pair + 1)
                pend = emit_attn(2 * pair + 1, pend)
                if pair == 2:
                    v1_strip = emit_v_load(1)
                    emit_v(1, v1_strip, list(range(0, 8)))
                elif pair == 3:
                    emit_v(1, v1_strip, list(range(8, SO)))
            pend()


            ps_u_ctx.__exit__(None, None, None)
            ps_qk_ctx.__exit__(None, None, None)
            evac_ctx.__exit__(None, None, None)
            exps_ctx.__exit__(None, None, None)
            wsmall_ctx.__exit__(None, None, None)
            ctx12.__exit__(None, None, None)
            evac_ctx = tc.tile_pool(name="evac2", bufs=3)
            evac_pool = evac_ctx.__enter__()
            ps_tr_ctx = tc.tile_pool(name="ps_tr2", bufs=2, space="PSUM")
            ps_tr = ps_tr_ctx.__enter__()

            # ---------------- phase 4: Wo projection + residual + LN2 ----------------
            for half in range(2):
                wo_strip = wstrip_pool.tile([P, KO, 512], BF16, tag="wstrip")
                nc.sync.dma_start(
                    wo_strip[:], wo.rearrange("(o p) n -> p o n", p=P)[:, :, half * 512:(half + 1) * 512])
                for m in range(4):
                    mm = half * 4 + m
                    psy = ps_w.tile([P, 512], F32, tag="ps_w")
                    for ko in range(KO):
                        nc.tensor.matmul(
                            psy[:], wo_strip[:, ko, m * P:(m + 1) * P], oT[:, ko, :],
                            start=(ko == 0), stop=(ko == KO - 1))
                    ysb = evac_pool.tile([P, 512], F32, tag="ysb")
                    nc.vector.tensor_scalar_add(out=ysb[:], in0=psy[:], scalar1=bo_pm[:, mm:mm + 1])
                    trp = ps_tr.tile([P, RT_OWN, P], F32, tag="tr")
                    for rt in range(RT_OWN):
                        nc.tensor.transpose(trp[:, rt, :], ysb[:, rt * P:(rt + 1) * P], id_f32[:])
                    nc.vector.tensor_tensor(
                        out=x1[:, :, mm * P:(mm + 1) * P],
                        in0=x1[:, :, mm * P:(mm + 1) * P], in1=trp[:], op=ALU.add)

            for rt in range(RT_OWN):
                y_row = hrow_pool.tile([P, D], BF16, tag="hrow")
                _layer_norm_rows(nc, sng, x1[:, rt, :], stats_pool, y_row[:])
                trp = ps_tr.tile([P, KO, P], BF16, tag="tr2")
                for ko in range(KO):
                    nc.tensor.transpose(trp[:, ko, :], y_row[:, ko * P:(ko + 1) * P], id_bf[:])
                nc.scalar.copy(out=h2T[:, :, rt * P:(rt + 1) * P], in_=trp[:])

            # ---------------- phase 5: FFN ----------------
            for half in range(2):
                w1_strip = wstrip_pool.tile([P, KO, 512], BF16, tag="wstrip")
                nc.sync.dma_start(
                    w1_strip[:], w1.rearrange("(o p) n -> p o n", p=P)[:, :, half * 512:(half + 1) * 512])
                for m in range(4):
                    mm = half * 4 + m
                    psf = ps_w.tile([P, 512], F32, tag="ps_w")
                    for ko in range(KO):
                        nc.tensor.matmul(
                            psf[:], w1_strip[:, ko, m * P:(m + 1) * P], h2T[:, ko, :],
                            start=(ko == 0), stop=(ko == KO - 1))
                    # f = gelu(x + b1), fused bias via activation
                    nc.scalar.activation(out=fT[:, mm, :], in_=psf[:], func=AF.Gelu,
                                         bias=bf1_pm[:, mm:mm + 1], scale=1.0)
            for half in range(2):
                w2_strip = wstrip_pool.tile([P, KO, 512], BF16, tag="wstrip")
                nc.sync.dma_start(
                    w2_strip[:], w2.rearrange("(o p) n -> p o n", p=P)[:, :, half * 512:(half + 1) * 512])
                for m in range(4):
                    mm = half * 4 + m
                    psz = ps_w.tile([P, 512], F32, tag="ps_w")
                    for ko in range(KO):
                        nc.tensor.matmul(
                            psz[:], w2_strip[:, ko, m * P:(m + 1) * P], fT[:, ko, :],
                            start=(ko == 0), stop=(ko == KO - 1))
                    zsb = evac_pool.tile([P, 512], F32, tag="ysb")
                    nc.vector.tensor_scalar_add(out=zsb[:], in0=psz[:], scalar1=bf2_pm[:, mm:mm + 1])
                    trp = ps_tr.tile([P, RT_OWN, P], F32, tag="tr")
                    for rt in range(RT_OWN):
                        nc.tensor.transpose(trp[:, rt, :], zsb[:, rt * P:(rt + 1) * P], id_f32[:])
                    nc.vector.tensor_tensor(
                        out=x1[:, :, mm * P:(mm + 1) * P],
                        in0=x1[:, :, mm * P:(mm + 1) * P], in1=trp[:], op=ALU.add)

            for rt in range(RT_OWN):
                nc.sync.dma_start(out[rt * P:(rt + 1) * P, :], x1[:, rt, :])

            ps_tr_ctx.__exit__(None, None, None)
            evac_ctx.__exit__(None, None, None)
            ps_w_ctx.__exit__(None, None, None)

    nc.compile()
    return nc


_NC_CACHE = None


def _get_nc():
    global _NC_CACHE
    if _NC_CACHE is None:
        _NC_CACHE = build_kernel()
    return _NC_CACHE


def _prep_weights(Wq, Wk, Wv, Wo, W1, W2, ln1_g, ln1_b, ln2_g, ln2_b, b1):
    """Fold LayerNorm gamma into the consuming weights and beta into bias
    vectors (exact math, done in f32 before the bf16 cast)."""
    bf = ml_dtypes.bfloat16
    # [H, D, E] -> [D, H*E]
    wq = np.ascontiguousarray(np.transpose(Wq, (1, 0, 2)).reshape(D, D))
    wk = np.ascontiguousarray(np.transpose(Wk, (1, 0, 2)).reshape(D, D))
    wv = np.ascontiguousarray(np.transpose(Wv, (1, 0, 2)).reshape(D, D))
    cq = ln1_b @ wq
    ck = ln1_b @ wk
    cv = ln1_b @ wv              # v bias; o = softmax(..)@v + cv, folded into bo
    bo_adj = cv @ Wo             # caller adds this to bo
    b1_adj = b1 + ln2_b @ W1
    return ((wq * ln1_g[:, None]).astype(bf), (wk * ln1_g[:, None]).astype(bf),
            (wv * ln1_g[:, None]).astype(bf), Wo.astype(bf),
            (W1 * ln2_g[:, None]).astype(bf), W2.astype(bf),
            cq.astype(np.float32), ck.astype(np.float32),
            bo_adj.astype(np.float32), b1_adj.astype(np.float32))


def kernel(x, Wq, Wk, Wv, Wo, bo, ln1_g, ln1_b, ln2_g, ln2_b, W1, b1, W2, b2,
           _trace=False):
    x = np.asarray(x, dtype=np.float32)
    wq, wk, wv, wo, w1, w2, cq_v, ck_v, bo_extra, b1_adj = _prep_weights(
        np.asarray(Wq, np.float32), np.asarray(Wk, np.float32),
        np.asarray(Wv, np.float32), np.asarray(Wo, np.float32),
        np.asarray(W1, np.float32), np.asarray(W2, np.float32),
        np.asarray(ln1_g, np.float32), np.asarray(ln1_b, np.float32),
        np.asarray(ln2_g, np.float32), np.asarray(ln2_b, np.float32),
        np.asarray(b1, np.float32))
    common = {
        "wq": wq, "wk": wk, "wv": wv, "wo": wo, "w1": w1, "w2": w2,
        "cq": cq_v, "ck": ck_v,
        "bo": np.asarray(bo, np.float32) + bo_extra, "b1": b1_adj,
        "b2": np.asarray(b2, np.float32),
    }
    in_maps = []
    for core in range(8):
        b, c = divmod(core, 4)
        xb_perm = np.concatenate(
            [x[b, c * TQ:(c + 1) * TQ], x[b, :c * TQ], x[b, (c + 1) * TQ:]], axis=0)
        in_maps.append({"xb": np.ascontiguousarray(xb_perm), **common})

    nc = _get_nc()
    res = run_bass_kernel_spmd(nc, in_maps, core_ids=list(range(8)), trace=_trace)
    out = np.empty((2, T, D), np.float32)
    for core in range(8):
        b, c = divmod(core, 4)
        out[b, c * TQ:(c + 1) * TQ] = res.results[core]["out"]
    if _trace:
        kernel.last_results = res
    return out


# revision 16
# speedup vs baseline: 1.2576x; 1.0390x over previous
"""Trainium2 Bass kernel for nn_Block_49624052138029 (dense transformer block).

Strategy: pure data parallelism across 8 NeuronCores. Core i handles batch
b = i//4 and query-chunk c = i%4 (512 of the 2048 tokens). The host permutes
each core's batch slice so its own 512 query rows come first; attention is
permutation-invariant over keys, so K/V row order doesn't matter. Each core
redundantly computes LN1 + K + V over all 2048 rows of its batch (cheaper
than on-chip collectives on this stack), and Q/attention/Wo/FFN only for its
own 512 rows.

On-chip layout: activations live feature-major ([D on partitions, tokens on
free]) for matmuls; LayerNorm runs row-major and the gamma/beta application is
fused into the PSUM-evacuation of the PE transpose (where D sits on
partitions). Attention computes S^T = K_h^T-chunks.T @ Q_h^T per head with an
exp() evacuation on ScalarE, and AV appends an all-ones column to V so the
softmax denominator falls out of the same accumulation (row 64 of U^T).

All matmul operands are bf16 (f32 PSUM accumulation); LN statistics,
residual stream and softmax denominators stay f32.
"""

import os
import sys

for _p in ("/root/.axon_site", "/root/.axon_site/_ro/trn_rl_repo",
           "/root/.axon_site/_ro/pypackages", "/opt/trn_rl_repo", "/opt/pypackages"):
    if os.path.isdir(_p) and _p not in sys.path:
        sys.path.append(_p)

import numpy as np
import ml_dtypes

import concourse.bass as bass
import concourse.tile as tile
from concourse import bacc, mybir
from concourse.bass_utils import run_bass_kernel_spmd
from concourse.masks import make_identity

F32 = mybir.dt.float32
BF16 = mybir.dt.bfloat16
FP8 = mybir.dt.float8e4
AF = mybir.ActivationFunctionType
ALU = mybir.AluOpType
AX = mybir.AxisListType
DRM = mybir.MatmulPerfMode.DoubleRow
NP_F8 = ml_dtypes.float8_e4m3
SW = 16.0         # fp8 weight scale for q/k/v projections

D = 1024          # model dim
H = 16            # heads
E = 64            # head dim
T = 2048          # tokens per batch
TQ = 512          # tokens owned by this core
P = 128
KO = D // P       # 8 feature chunks
RT_ALL = T // P   # 16 row tiles per batch
RT_OWN = TQ // P  # 4 row tiles owned
SO = T // P       # 16 key chunks
EPS = 1e-5
SCALE = 1.0 / 32.0  # D ** -0.5


def _layer_norm_rows(nc, sng, xrow_ap, stats_pool, y_out_ap, y_eng="dve"):
    """Row-major LayerNorm core: y = (x - mean(x)) * rsqrt(var(x) + eps).

    xrow_ap: [128, 1024] f32 SBUF; y_out_ap: [128, 1024] (any dtype) SBUF.
    gamma/beta are NOT applied here (folded into the weights host-side).
    y_eng picks the engine for the [128,1024] normalize op so callers can
    balance DVE vs ACT load per phase.
    """
    stats = stats_pool.tile([P, 2, 6], F32, tag="bnstats")
    xg = xrow_ap.rearrange("p (g d) -> p g d", g=2)
    for g in range(2):
        nc.vector.bn_stats(out=stats[:, g, :], in_=xg[:, g, :])
    mv = stats_pool.tile([P, 2], F32, tag="bnaggr")
    nc.vector.bn_aggr(out=mv[:], in_=stats[:])
    rstd = stats_pool.tile([P, 1], F32, tag="rstd")
    # rstd = 1 / sqrt(var + eps)
    nc.scalar.activation(out=rstd[:], in_=mv[:, 1:2], func=AF.Sqrt,
                         bias=sng["eps"][:], scale=1.0)
    nc.vector.reciprocal(out=rstd[:], in_=rstd[:])
    if y_eng == "act":
        # y = x*rstd + (-mean*rstd) on ScalarE
        nmr = stats_pool.tile([P, 1], F32, tag="nmr")
        nc.vector.tensor_scalar(
            out=nmr[:], in0=mv[:, 0:1], scalar1=rstd[:], scalar2=-1.0,
            op0=ALU.mult, op1=ALU.mult)
        nc.scalar.activation(out=y_out_ap, in_=xrow_ap, func=AF.Identity,
                             scale=rstd[:], bias=nmr[:])
    else:
        nc.vector.tensor_scalar(
            out=y_out_ap, in0=xrow_ap, scalar1=mv[:, 0:1], scalar2=rstd[:],
            op0=ALU.subtract, op1=ALU.mult)


def build_kernel():
    nc = bacc.Bacc(None, target_bir_lowering=False, debug=False)

    xb = nc.dram_tensor("xb", [T, D], F32, kind="ExternalInput")
    wq = nc.dram_tensor("wq", [D, D], FP8, kind="ExternalInput")
    wk = nc.dram_tensor("wk", [D, D], FP8, kind="ExternalInput")
    wv = nc.dram_tensor("wv", [D, D], FP8, kind="ExternalInput")
    wo = nc.dram_tensor("wo", [D, D], BF16, kind="ExternalInput")
    w1 = nc.dram_tensor("w1", [D, D], BF16, kind="ExternalInput")
    w2 = nc.dram_tensor("w2", [D, D], BF16, kind="ExternalInput")
    cq = nc.dram_tensor("cq", [D], F32, kind="ExternalInput")
    ck = nc.dram_tensor("ck", [D], F32, kind="ExternalInput")
    bo = nc.dram_tensor("bo", [D], F32, kind="ExternalInput")
    b1 = nc.dram_tensor("b1", [D], F32, kind="ExternalInput")
    b2 = nc.dram_tensor("b2", [D], F32, kind="ExternalInput")
    out = nc.dram_tensor("out", [TQ, D], F32, kind="ExternalOutput")

    # per-feature params as [128, 8] (partition p, chunk o) for feature-major use
    def pm(dram_vec):
        return dram_vec.rearrange("(o p) -> p o", p=P)

    def wdr(w):     # [D, D] -> [p, a, t, n] DoubleRow stationary view
        return w.rearrange("(a t p) n -> p a t n", t=2, p=P)

    with tile.TileContext(nc) as tc:
        with (
            tc.tile_pool(name="singles", bufs=1) as singles,
            tc.tile_pool(name="persist", bufs=1) as persist,
            tc.tile_pool(name="hrow", bufs=4) as hrow_pool,
            tc.tile_pool(name="stats", bufs=6) as stats_pool,
            tc.tile_pool(name="wstrip", bufs=2) as wstrip_pool,
        ):
            # ---------------- setup ----------------
            sng = {}
            id_bf = singles.tile([P, P], BF16, name="id_bf")
            make_identity(nc, id_bf[:])
            id_f32 = singles.tile([P, P], F32, name="id_f32")
            make_identity(nc, id_f32[:])
            sng["eps"] = singles.tile([P, 1], F32, name="eps")
            nc.vector.memset(sng["eps"][:], EPS)
            ones64 = singles.tile([1, E], BF16, name="ones64")
            nc.vector.memset(ones64[:], 1.0)


            cq_pm = singles.tile([P, KO], F32, name="cq_pm")
            nc.sync.dma_start(cq_pm[:], pm(cq))
            ck_pm = singles.tile([P, KO], F32, name="ck_pm")
            nc.sync.dma_start(ck_pm[:], pm(ck))
            bo_pm = singles.tile([P, KO], F32, name="bo_pm")
            nc.sync.dma_start(bo_pm[:], pm(bo))
            bf1_pm = singles.tile([P, KO], F32, name="bf1_pm")
            nc.sync.dma_start(bf1_pm[:], pm(b1))
            bf2_pm = singles.tile([P, KO], F32, name="bf2_pm")
            nc.sync.dma_start(bf2_pm[:], pm(b2))

            # ---------------- persistent activations ----------------
            kT = persist.tile([P, KO, T], BF16, name="kT")          # 4 MB
            vP = persist.tile([P, SO // 2, 2, H, E + 1], FP8, name="vP")  # 2.08 MB
            qPack = persist.tile([P, KO, 2, TQ], BF16, name="qPack")  # 2 MB
            nc.vector.memset(qPack[:], 0.0)
            oT = persist.tile([P, KO, TQ], BF16, name="oT")         # 1 MB
            x1 = persist.tile([P, RT_OWN, D], F32, name="x1")       # 2 MB
            h2T = persist.tile([P, KO, TQ], BF16, name="h2T")       # 1 MB
            fT = persist.tile([P, KO, TQ], BF16, name="fT")         # 1 MB

            # ones column of vP (softmax denominator trick)
            nc.vector.memset(vP[:, :, :, :, E], 1.0)

            # ---------------- phase 1: LN1 over all rows -> hT ----------------
            ps_w_ctx = tc.tile_pool(name="ps_w", bufs=2, space="PSUM")
            ps_w = ps_w_ctx.__enter__()
            ps_tr_ctx = tc.tile_pool(name="ps_tr", bufs=2, space="PSUM")
            ps_tr = ps_tr_ctx.__enter__()
            ctx12 = tc.tile_pool(name="hTp", bufs=1)
            hT_pool = ctx12.__enter__()
            wsmall_ctx = tc.tile_pool(name="wsmall", bufs=3)
            wsmall_pool = wsmall_ctx.__enter__()
            xrow_ctx = tc.tile_pool(name="xrow", bufs=3)
            xrow_pool = xrow_ctx.__enter__()
            hT = hT_pool.tile([P, KO, T], FP8, name="hT")           # 2 MB
            for rt in range(RT_ALL):
                if rt < RT_OWN:
                    x_t = x1[:, rt, :]   # own rows: keep the raw x for residual
                    nc.sync.dma_start(x_t, xb[rt * P:(rt + 1) * P, :])
                else:
                    x_tile = xrow_pool.tile([P, D], F32, tag="xrow")
                    nc.sync.dma_start(x_tile[:], xb[rt * P:(rt + 1) * P, :])
                    x_t = x_tile[:]
                y_row = hrow_pool.tile([P, D], BF16, tag="hrow")
                _layer_norm_rows(nc, sng, x_t, stats_pool, y_row[:])
                # gamma/beta are folded into the weights host-side, so the
                # transpose evacuation is a plain copy (batched 2 chunks/op)
                trp = ps_tr.tile([P, KO, P], BF16, tag="tr")
                for ko in range(KO):
                    nc.tensor.transpose(trp[:, ko, :], y_row[:, ko * P:(ko + 1) * P], id_bf[:])
                nc.scalar.copy(out=hT[:, :, rt * P:(rt + 1) * P], in_=trp[:])

            # ---------------- phases 2+3: q/k/v interleaved with attention ----------------
            xrow_ctx.__exit__(None, None, None)
            ps_tr_ctx.__exit__(None, None, None)
            exps_ctx = tc.tile_pool(name="exps", bufs=8)
            exps_pool = exps_ctx.__enter__()
            evac_ctx = tc.tile_pool(name="evac", bufs=4)
            evac_pool = evac_ctx.__enter__()
            ps_qk_ctx = tc.tile_pool(name="ps_qk", bufs=2, space="PSUM")
            ps_qk = ps_qk_ctx.__enter__()
            ps_u_ctx = tc.tile_pool(name="ps_u", bufs=2, space="PSUM")
            ps_u = ps_u_ctx.__enter__()

            # Dense k/v matmuls are emitted right after each attention head
            # pair (lower scheduler priority), so the in-order PE fills
            # exp-latency gaps with dense work instead of idling.
            def emit_k(he):
                wk_he = wsmall_pool.tile([P, 4, 2, P], FP8, tag="w_he", name="wk_he")
                nc.sync.dma_start(
                    wk_he[:], wdr(wk)[:, :, :, he * P:(he + 1) * P])
                for t in range(RT_ALL // 4):
                    psk = ps_w.tile([P, 512], F32, tag="ps_w", name="psk")
                    for a in range(4):
                        nc.tensor.matmul(
                            psk[:], wk_he[:, a], hT[:, 2 * a:2 * a + 2, t * 512:(t + 1) * 512],
                            start=(a == 0), stop=(a == 3), perf_mode=DRM)
                    nc.vector.tensor_scalar(
                        out=kT[:, he, t * 512:(t + 1) * 512], in0=psk[:],
                        scalar1=1.0 / SW, scalar2=ck_pm[:, he:he + 1],
                        op0=ALU.mult, op1=ALU.add)

            def emit_q(he):
                wq_he = wsmall_pool.tile([P, 4, 2, P], FP8, tag="w_he", name="wq_he")
                nc.sync.dma_start(
                    wq_he[:], wdr(wq)[:, :, :, he * P:(he + 1) * P])
                psq = ps_w.tile([P, 512], F32, tag="ps_w", name="psq")
                for a in range(4):
                    nc.tensor.matmul(
                        psq[:], wq_he[:, a], hT[:, 2 * a:2 * a + 2, 0:TQ],
                        start=(a == 0), stop=(a == 3), perf_mode=DRM)
                nc.vector.tensor_scalar(
                    out=qPack[0:E, he, 0, :], in0=psq[0:E, :],
                    scalar1=1.0 / SW, scalar2=cq_pm[0:E, he:he + 1],
                    op0=ALU.mult, op1=ALU.add)
                nc.vector.tensor_scalar(
                    out=qPack[E:P, he, 1, :], in0=psq[E:P, :],
                    scalar1=1.0 / SW, scalar2=cq_pm[E:P, he:he + 1],
                    op0=ALU.mult, op1=ALU.add)

            def emit_v_load(nh):
                wv_strip = wstrip_pool.tile([P, 4, 2, 512], FP8, tag="wstrip8", name="wv_strip")
                nc.sync.dma_start(
                    wv_strip[:], wdr(wv)[:, :, :, nh * 512:(nh + 1) * 512])
                return wv_strip

            def emit_v(nh, wv_strip=None, so_list=None, evac_eng=None):
                if wv_strip is None:
                    wv_strip = emit_v_load(nh)
                for so in (range(SO) if so_list is None else so_list):
                    psv = ps_w.tile([P, 512], F32, tag="ps_w", name="psv")
                    for a in range(4):
                        nc.tensor.matmul(
                            psv[:], hT[:, 2 * a:2 * a + 2, so * P:(so + 1) * P],
                            wv_strip[:, a], start=(a == 0), stop=(a == 3),
                            perf_mode=DRM)
                    nc.vector.tensor_scalar(
                        out=vP[:, so // 2, so % 2, nh * 8:(nh + 1) * 8, 0:E],
                        in0=psv[:].rearrange("p (h e) -> p h e", e=E),
                        scalar1=1.0 / SW, scalar2=None, op0=ALU.mult)

            def emit_attn(h, pend_epi=None):
                # Returns this head's deferred epilogue closure. The caller
                # passes the PREVIOUS head's closure, flushed mid-way through
                # this head's QK stream so the in-order PE never waits on the
                # DVE reciprocal chain (was a ~2.5us stall per head).
                pbase = (h % 2) * E
                ko_h = h // 2
                psu = ps_u.tile([P, 512], F32, tag="ps_u", name="psu")
                for sp in range(SO // 2):
                    if sp == 3 and pend_epi is not None:
                        pend_epi()
                        pend_epi = None
                    pss = ps_qk.tile([P, 2, 512], F32, tag="ps_qk", name="pss")
                    for j in range(2):
                        so = 2 * sp + j
                        # full-K stationary (FWL-eligible); the other head's
                        # rows meet zeros in the packed q, so the sum is exact
                        nc.tensor.matmul(
                            pss[:, j, :],
                            kT[:, ko_h, so * P:(so + 1) * P],
                            qPack[:, ko_h, h % 2, :],
                            start=True, stop=True)
                    es = exps_pool.tile([P, 2, 512], FP8, tag="exps", name="es")
                    nc.scalar.activation(out=es[:], in_=pss[:], func=AF.Exp, scale=SCALE)
                    # fp8 DoubleRow: virtual K=256 sums both key chunks at once
                    nc.tensor.matmul(
                        psu[0:E + 1, :], vP[:, sp, :, h, :], es[:],
                        start=(sp == 0), stop=(sp == SO // 2 - 1),
                        perf_mode=mybir.MatmulPerfMode.DoubleRow)
                dinv_f = stats_pool.tile([1, TQ], F32, tag="dinv_f", name="dinv_f")
                nc.vector.reciprocal(out=dinv_f[:], in_=psu[E:E + 1, :])
                dinv = stats_pool.tile([1, TQ], BF16, tag="dinv", name="dinv")
                nc.vector.tensor_copy(out=dinv[:], in_=dinv_f[:])

                def epi():
                    psb = ps_qk.tile([P, 2, 512], F32, tag="ps_qk", name="psb")
                    nc.tensor.matmul(psb[0:E, 0, :], ones64[:], dinv[:],
                                     start=True, stop=True)
                    dbc = evac_pool.tile([E, 512], BF16, tag="dbc", name="dbc")
                    nc.vector.tensor_copy(out=dbc[:], in_=psb[0:E, 0, :])
                    nc.vector.tensor_tensor(
                        out=oT[pbase:pbase + E, ko_h, :], in0=psu[0:E, :],
                        in1=dbc[:], op=ALU.mult)
                return epi

            for he in range(KO):
                emit_q(he)
            emit_k(0)
            emit_v(0)
            pend = None
            for pair in range(KO):
                pend = emit_attn(2 * pair, pend)
                if pair + 1 < KO:
                    emit_k(pair + 1)
                pend = emit_attn(2 * pair + 1, pend)
                if pair == 2:
                    v1_strip = emit_v_load(1)
                    emit_v(1, v1_strip, list(range(0, 8)))
                elif pair == 3:
                    emit_v(1, v1_strip, list(range(8, SO)))
            pend()


            ps_u_ctx.__exit__(None, None, None)
            ps_qk_ctx.__exit__(None, None, None)
            evac_ctx.__exit__(None, None, None)
            exps_ctx.__exit__(None, None, None)
            wsmall_ctx.__exit__(None, None, None)
            ctx12.__exit__(None, None, None)
            evac_ctx = tc.tile_pool(name="evac2", bufs=3)
            evac_pool = evac_ctx.__enter__()
            ps_tr_ctx = tc.tile_pool(name="ps_tr2", bufs=2, space="PSUM")
            ps_tr = ps_tr_ctx.__enter__()

            # ---------------- phase 4: Wo projection + residual + LN2 ----------------
            for half in range(2):
                wo_strip = wstrip_pool.tile([P, KO, 512], BF16, tag="wstrip")
                nc.sync.dma_start(
                    wo_strip[:], wo.rearrange("(o p) n -> p o n", p=P)[:, :, half * 512:(half + 1) * 512])
                for m in range(4):
                    mm = half * 4 + m
                    psy = ps_w.tile([P, 512], F32, tag="ps_w")
                    for ko in range(KO):
                        nc.tensor.matmul(
                            psy[:], wo_strip[:, ko, m * P:(m + 1) * P], oT[:, ko, :],
                            start=(ko == 0), stop=(ko == KO - 1))
                    ysb = evac_pool.tile([P, 512], F32, tag="ysb")
                    nc.vector.tensor_scalar_add(out=ysb[:], in0=psy[:], scalar1=bo_pm[:, mm:mm + 1])
                    trp = ps_tr.tile([P, RT_OWN, P], F32, tag="tr")
                    for rt in range(RT_OWN):
                        nc.tensor.transpose(trp[:, rt, :], ysb[:, rt * P:(rt + 1) * P], id_f32[:])
                    nc.vector.tensor_tensor(
                        out=x1[:, :, mm * P:(mm + 1) * P],
                        in0=x1[:, :, mm * P:(mm + 1) * P], in1=trp[:], op=ALU.add)

            for rt in range(RT_OWN):
                y_row = hrow_pool.tile([P, D], BF16, tag="hrow")
                _layer_norm_rows(nc, sng, x1[:, rt, :], stats_pool, y_row[:])
                trp = ps_tr.tile([P, KO, P], BF16, tag="tr2")
                for ko in range(KO):
                    nc.tensor.transpose(trp[:, ko, :], y_row[:, ko * P:(ko + 1) * P], id_bf[:])
                nc.scalar.copy(out=h2T[:, :, rt * P:(rt + 1) * P], in_=trp[:])

            # ---------------- phase 5: FFN ----------------
            for half in range(2):
                w1_strip = wstrip_pool.tile([P, KO, 512], BF16, tag="wstrip")
                nc.sync.dma_start(
                    w1_strip[:], w1.rearrange("(o p) n -> p o n", p=P)[:, :, half * 512:(half + 1) * 512])
                for m in range(4):
                    mm = half * 4 + m
                    psf = ps_w.tile([P, 512], F32, tag="ps_w")
                    for ko in range(KO):
                        nc.tensor.matmul(
                            psf[:], w1_strip[:, ko, m * P:(m + 1) * P], h2T[:, ko, :],
                            start=(ko == 0), stop=(ko == KO - 1))
                    # f = gelu(x + b1), fused bias via activation
                    nc.scalar.activation(out=fT[:, mm, :], in_=psf[:], func=AF.Gelu,
                                         bias=bf1_pm[:, mm:mm + 1], scale=1.0)
            for half in range(2):
                w2_strip = wstrip_pool.tile([P, KO, 512], BF16, tag="wstrip")
                nc.sync.dma_start(
                    w2_strip[:], w2.rearrange("(o p) n -> p o n", p=P)[:, :, half * 512:(half + 1) * 512])
                for m in range(4):
                    mm = half * 4 + m
                    psz = ps_w.tile([P, 512], F32, tag="ps_w")
                    for ko in range(KO):
                        nc.tensor.matmul(
                            psz[:], w2_strip[:, ko, m * P:(m + 1) * P], fT[:, ko, :],
                            start=(ko == 0), stop=(ko == KO - 1))
                    zsb = evac_pool.tile([P, 512], F32, tag="ysb")
                    nc.vector.tensor_scalar_add(out=zsb[:], in0=psz[:], scalar1=bf2_pm[:, mm:mm + 1])
                    trp = ps_tr.tile([P, RT_OWN, P], F32, tag="tr")
                    for rt in range(RT_OWN):
                        nc.tensor.transpose(trp[:, rt, :], zsb[:, rt * P:(rt + 1) * P], id_f32[:])
                    nc.vector.tensor_tensor(
                        out=x1[:, :, mm * P:(mm + 1) * P],
                        in0=x1[:, :, mm * P:(mm + 1) * P], in1=trp[:], op=ALU.add)

            for rt in range(RT_OWN):
                nc.sync.dma_start(out[rt * P:(rt + 1) * P, :], x1[:, rt, :])

            ps_tr_ctx.__exit__(None, None, None)
            evac_ctx.__exit__(None, None, None)
            ps_w_ctx.__exit__(None, None, None)

    nc.compile()
    return nc


_NC_CACHE = None


def _get_nc():
    global _NC_CACHE
    if _NC_CACHE is None:
        _NC_CACHE = build_kernel()
    return _NC_CACHE


def _prep_weights(Wq, Wk, Wv, Wo, W1, W2, ln1_g, ln1_b, ln2_g, ln2_b, b1):
    """Fold LayerNorm gamma into the consuming weights and beta into bias
    vectors (exact math, done in f32 before the bf16 cast)."""
    bf = ml_dtypes.bfloat16
    # [H, D, E] -> [D, H*E]
    wq = np.ascontiguousarray(np.transpose(Wq, (1, 0, 2)).reshape(D, D))
    wk = np.ascontiguousarray(np.transpose(Wk, (1, 0, 2)).reshape(D, D))
    wv = np.ascontiguousarray(np.transpose(Wv, (1, 0, 2)).reshape(D, D))
    cq = ln1_b @ wq
    ck = ln1_b @ wk
    cv = ln1_b @ wv              # v bias; o = softmax(..)@v + cv, folded into bo
    bo_adj = cv @ Wo             # caller adds this to bo
    b1_adj = b1 + ln2_b @ W1
    return (np.ascontiguousarray(wq * ln1_g[:, None] * SW).astype(NP_F8),
            np.ascontiguousarray(wk * ln1_g[:, None] * SW).astype(NP_F8),
            np.ascontiguousarray(wv * ln1_g[:, None] * SW).astype(NP_F8),
            Wo.astype(bf),
            (W1 * ln2_g[:, None]).astype(bf), W2.astype(bf),
            cq.astype(np.float32), ck.astype(np.float32),
            bo_adj.astype(np.float32), b1_adj.astype(np.float32))


def kernel(x, Wq, Wk, Wv, Wo, bo, ln1_g, ln1_b, ln2_g, ln2_b, W1, b1, W2, b2,
           _trace=False):
    x = np.asarray(x, dtype=np.float32)
    wq, wk, wv, wo, w1, w2, cq_v, ck_v, bo_extra, b1_adj = _prep_weights(
        np.asarray(Wq, np.float32), np.asarray(Wk, np.float32),
        np.asarray(Wv, np.float32), np.asarray(Wo, np.float32),
        np.asarray(W1, np.float32), np.asarray(W2, np.float32),
        np.asarray(ln1_g, np.float32), np.asarray(ln1_b, np.float32),
        np.asarray(ln2_g, np.float32), np.asarray(ln2_b, np.float32),
        np.asarray(b1, np.float32))
    common = {
        "wq": wq, "wk": wk, "wv": wv, "wo": wo, "w1": w1, "w2": w2,
        "cq": cq_v, "ck": ck_v,
        "bo": np.asarray(bo, np.float32) + bo_extra, "b1": b1_adj,
        "b2": np.asarray(b2, np.float32),
    }
    in_maps = []
    for core in range(8):
        b, c = divmod(core, 4)
        xb_perm = np.concatenate(
            [x[b, c * TQ:(c + 1) * TQ], x[b, :c * TQ], x[b, (c + 1) * TQ:]], axis=0)
        in_maps.append({"xb": np.ascontiguousarray(xb_perm), **common})

    nc = _get_nc()
    res = run_bass_kernel_spmd(nc, in_maps, core_ids=list(range(8)), trace=_trace)
    out = np.empty((2, T, D), np.float32)
    for core in range(8):
        b, c = divmod(core, 4)
        out[b, c * TQ:(c + 1) * TQ] = res.results[core]["out"]
    if _trace:
        kernel.last_results = res
    return out
